# revision 30
# baseline (speedup 1.0000x reference)
"""Trainium2 Bass kernel for nn_BailingMoeBlock (8 NeuronCores).

Sharding (v5):
  - rmsnorm1: transposes run on RAW resid (bf16); 1/rms folded into the
    rope cos/sin tables and the v copy, so the PE never waits on the
    variance reduction.
  - Attention: token-parallel (own 128 q x all 1024 kv); single kv
    AllGather.  Softmax: additive causal mask (-4 exp-bias folded in, -60
    masked), exp with accum_out row-sums, DVE renormalize.  Heads run in
    kv-sharing pairs; pair context = one N=256 matmul per kv block;
    depth-2 head pipeline.  All staging DMAs stay on the sync/scalar
    hardware-DGE queues (gpsimd SWDGE staging faults on this runtime).
  - resid2 / rmsnorm2 / router on the owned block; x2^T + router weights
    leave in two chunked AllGathers (hc 0..9, then hc 10..15 + comb).
  - MoE: expert-parallel (2 experts/core), hc-outer/half-inner loops so
    each LDWEIGHTS serves two N=512 matmuls; shared expert
    column-parallel.  w2 emits 5 uneven column groups
    (512,512,512,384,128) -> 5 AllToAlls; the tiny last group shrinks the
    exposed tail.  Local 8-way adds via identity matmuls.

Heavy matmuls in bf16 (fp32 accumulate in PSUM); router in fp32.
"""
import sys
import numpy as np

for _p in ("/opt/trn_rl_repo", "/opt/pypackages"):
    if _p not in sys.path:
        sys.path.append(_p)

import ml_dtypes  # noqa: E402
from concourse import bacc  # noqa: E402
import concourse.bass as bass  # noqa: E402
import concourse.tile as tile  # noqa: E402
import concourse.mybir as mybir  # noqa: E402
from concourse.bass_utils import run_bass_kernel_spmd  # noqa: E402
from concourse.masks import make_identity  # noqa: E402

F32 = mybir.dt.float32
BF16 = mybir.dt.bfloat16
BF16_NP = ml_dtypes.bfloat16

N_CORES = 8
T = 1024
H = 2048
NH = 16
NKV = 4
D = 128
E = 16
I_EXP = 512
SI = 1024
TOPK = 4
RMS_EPS = 1e-6
ROPE_THETA = 10000.0
Q_SIZE = NH * D          # 2048
KV_SIZE = NKV * D        # 512
TB = T // 128            # 8 token blocks
HC = H // 128            # 16 h chunks
NEG_BIG = -30000.0
HCA = 10                 # x2T chunks in first AG1 chunk
HCB = HC - HCA           # remaining chunks (ride with router weights)
W2G = (512, 512, 512, 384, 128)   # w2 column-group widths
W2OFF = (0, 512, 1024, 1536, 1920)

RS2_DT = BF16

X = mybir.AxisListType.X
ALU = mybir.AluOpType
ACTF = mybir.ActivationFunctionType


def build_nc(debug=False):
    nc = bacc.Bacc("TRN2", target_bir_lowering=False, debug=False,
                   num_devices=N_CORES)

    def din(name, shape, dt):
        return nc.dram_tensor(name, list(shape), dt, kind="ExternalInput").ap()

    def dout(name, shape, dt):
        return nc.dram_tensor(name, list(shape), dt, kind="ExternalOutput").ap()

    wkv_d = din("wkv", (HC, 128, 1024), BF16)    # k|v columns of w_qkv
    wq_d = din("wq", (HC, 128, 2048), BF16)      # q columns of w_qkv
    wd_d = din("wdense", (HC, 128, H), BF16)     # full w_dense, chunk-major
    sw13_d = din("sw13", (128, HC, 256), BF16)
    sw2_d = din("sw2", (128, H), BF16)
    w13a_d = din("w13a", (8, 128, H), BF16)   # [cc][p][hc*128+ci]
    w13b_d = din("w13b", (8, 128, H), BF16)
    w2a_d = din("w2a", (128, 4, H), BF16)
    w2b_d = din("w2b", (128, 4, H), BF16)
    gate_d = din("gate", (128, HC, E), F32)
    resid_own_d = din("resid_own", (128, H), F32)  # host hid+res own block
    qcos_d = din("qcos", (128, 8, 64), F32)   # own-token cos*scale, x8 heads
    qsin_d = din("qsin", (128, 8, 64), F32)
    kcos_d = din("kcos", (128, 4, 64), F32)
    ksin_d = din("ksin", (128, 4, 64), F32)
    maskb_d = din("maskb", (128, T), BF16)    # additive mask: -4 / -30004
    sela_d = din("sela", (E, 128), F32)
    selb_d = din("selb", (E, 128), F32)

    out0_d = dout("out0", (128, H), F32)
    out1_d = dout("out1", (128, H), F32)
    dbg = {}
    if debug:
        dbg["q"] = dout("dbg_q", (128, 128), F32)        # qT head0 (own toks)
        dbg["att"] = dout("dbg_att", (128, H), F32)      # attn_out own block
        dbg["x2"] = dout("dbg_x2", (128, H), F32)        # x2 own block fp32
        dbg["comb"] = dout("dbg_comb", (128, E), F32)    # comb own block
        dbg["acta"] = dout("dbg_acta", (128, 4 * T), F32)  # act expert a

    with tile.TileContext(nc) as tc:
        with (
            tc.tile_pool(name="const", bufs=1) as pc,
            tc.tile_pool(name="weights", bufs=1) as pw,
            tc.tile_pool(name="big", bufs=1) as pbig,
            tc.tile_pool(name="stream", bufs=3) as pstream,
            tc.tile_pool(name="tmp", bufs=2) as ptmp,
            tc.tile_pool(name="psA", bufs=3, space="PSUM") as psA,
            tc.tile_pool(name="psB", bufs=2, space="PSUM") as psB,
            tc.tile_pool(name="dram", bufs=1, space="DRAM") as pd,
        ):
            # ---------------- constants (scalar/ACT DMA queue) -------------
            ident_b = pc.tile([128, 128], BF16, name="ident_b")
            make_identity(nc, ident_b[:])
            ident_f = pc.tile([128, 128], F32, name="ident_f")
            make_identity(nc, ident_f[:])
            qcos = pc.tile([128, 8, 64], F32, name="qcos")
            nc.scalar.dma_start(qcos[:], qcos_d[:])
            qsin = pc.tile([128, 8, 64], F32, name="qsin")
            nc.scalar.dma_start(qsin[:], qsin_d[:])
            kcos = pc.tile([128, 4, 64], F32, name="kcos")
            nc.scalar.dma_start(kcos[:], kcos_d[:])
            ksin = pc.tile([128, 4, 64], F32, name="ksin")
            nc.scalar.dma_start(ksin[:], ksin_d[:])
            maskb = pc.tile([128, T], BF16, name="maskb")
            nc.scalar.dma_start(maskb[:], maskb_d[:])
            gate_sb = pc.tile([128, HC, E], F32, name="gate_sb")
            nc.scalar.dma_start(gate_sb[:], gate_d[:])
            sela_sb = pc.tile([E, 128], F32, name="sela_sb")
            nc.scalar.dma_start(sela_sb[:], sela_d[:])
            selb_sb = pc.tile([E, 128], F32, name="selb_sb")
            nc.scalar.dma_start(selb_sb[:], selb_d[:])

            # ---------------- DRAM internal buffers ----------------
            agkv_in = pd.tile([128, 1024], BF16, name="agkv_in")
            agkv_out = pd.tile([N_CORES, 128, 1024], BF16, name="agkv_out",
                               addr_space="Shared")
            ag1a_in = pd.tile([128, HCA * 128], BF16, name="ag1a_in")
            ag1a_out = pd.tile([TB, 128, HCA * 128], BF16,
                               name="ag1a_out", addr_space="Shared")
            ag1b_in = pd.tile([128, HCB * 128 + E], BF16, name="ag1b_in")
            ag1b_out = pd.tile([TB, 128, HCB * 128 + E], BF16,
                               name="ag1b_out", addr_space="Shared")
            rs2_in = [pd.tile([T, 1024], RS2_DT, name=f"rs2_in{g}")
                      for g in range(2)]
            a2a2_out = [pd.tile([TB, 128, 1024], RS2_DT,
                                name=f"a2a2_out{g}")
                        for g in range(2)]
            rg = [list(range(N_CORES))]

            # ===== P0: own-block resid; transposes on RAW resid =====
            resid_own = pbig.tile([128, H], F32, name="resid_own", tag="ro")
            nc.sync.dma_start(resid_own[:, 0:1024], resid_own_d[:, 0:1024])
            nc.sync.dma_start(resid_own[:, 1024:2048],
                              resid_own_d[:, 1024:2048])
            rb = ptmp.tile([128, H], BF16, name="rb", tag="x", bufs=2)
            nc.scalar.activation(rb[:, 0:1024], resid_own[:, 0:1024],
                                 ACTF.Copy)
            nc.scalar.activation(rb[:, 1024:2048], resid_own[:, 1024:2048],
                                 ACTF.Copy)
            # variance (off the transpose critical path)
            sqj = ptmp.tile([128, H], BF16, name="sqj", tag="x", bufs=2)
            ssum0 = ptmp.tile([128, 1], F32, name="ssum0", tag="nrm", bufs=4)
            nc.scalar.activation(sqj[:], resid_own[:], ACTF.Square,
                                 accum_out=ssum0[:])
            var0 = ptmp.tile([128, 1], F32, name="var0", tag="nrm", bufs=4)
            nc.vector.tensor_scalar(var0[:], ssum0[:], 1.0 / H, RMS_EPS,
                                    ALU.mult, ALU.add)
            sd0 = ptmp.tile([128, 1], F32, name="sd0", tag="nrm", bufs=4)
            nc.scalar.activation(sd0[:], var0[:], ACTF.Sqrt)
            rstd0 = ptmp.tile([128, 1], F32, name="rstd0", tag="nrm", bufs=4)
            nc.vector.reciprocal(rstd0[:], sd0[:])
            # rstd-folded rope tables (one tile; overlays P6's cb slot)
            cosR_all = ptmp.tile([128, 24, 64], F32, name="cosR_all",
                                 tag="cb", bufs=1)
            qcosR = cosR_all[:, 0:8, :]
            qsinR = cosR_all[:, 8:16, :]
            kcosR = cosR_all[:, 16:20, :]
            ksinR = cosR_all[:, 20:24, :]
            nc.vector.tensor_scalar_mul(qcosR, qcos[:], rstd0[:])
            nc.vector.tensor_scalar_mul(qsinR, qsin[:], rstd0[:])
            nc.vector.tensor_scalar_mul(kcosR, kcos[:], rstd0[:])
            nc.vector.tensor_scalar_mul(ksinR, ksin[:], rstd0[:])

            xT_raw = ptmp.tile([128, HC, 128], BF16, name="xT_raw",
                               tag="xTown", bufs=1)
            for hg in range(4):
                tp = psB.tile([128, 4, 128], BF16, name=f"tpx_{hg}", tag="B")
                for j in range(4):
                    hcc = hg * 4 + j
                    nc.tensor.transpose(
                        tp[:, j, :], rb[:, hcc * 128:(hcc + 1) * 128],
                        ident_b[:])
                if hg % 2 == 0:
                    nc.vector.tensor_copy(xT_raw[:, hg * 4:(hg + 1) * 4, :],
                                          tp[:])
                else:
                    nc.scalar.activation(xT_raw[:, hg * 4:(hg + 1) * 4, :],
                                         tp[:], ACTF.Copy)

            # ===== P1a: kv projection (raw) + rstd-folded rope + AGk/AGv ===
            pskv = psA.tile([128, 1024], F32, name="pskv", tag="A")
            for hc in range(HC):
                wkvc = pstream.tile([128, 1024], BF16, name=f"wkv_{hc}",
                                    tag="wstr", bufs=4)
                (nc.sync if hc % 2 == 0 else nc.scalar).dma_start(
                    wkvc[:], wkv_d[hc])
                for c2 in range(2):
                    nc.tensor.matmul(
                        pskv[:, c2 * 512:(c2 + 1) * 512], xT_raw[:, hc, :],
                        wkvc[:, c2 * 512:(c2 + 1) * 512],
                        start=(hc == 0), stop=(hc == HC - 1))

            def rope_tok(pview, cost, sint, dst, nh):
                """pview [128, nh, 128] psum; dst [128, nh, 128] bf16 sbuf."""
                x1 = pview[:, :, 0:64]
                x2 = pview[:, :, 64:128]
                ta = ptmp.tile([128, nh, 64], F32, name="ta", tag="rope1",
                               bufs=2)
                tb = ptmp.tile([128, nh, 64], F32, name="tb", tag="rope2",
                               bufs=2)
                tc2 = ptmp.tile([128, nh, 64], F32, name="tc", tag="rope1",
                                bufs=2)
                td = ptmp.tile([128, nh, 64], F32, name="td", tag="rope2",
                               bufs=2)
                nc.vector.tensor_tensor(ta[:], x1, cost, ALU.mult)
                nc.vector.tensor_tensor(tb[:], x2, sint, ALU.mult)
                nc.vector.tensor_tensor(tc2[:], x2, cost, ALU.mult)
                nc.vector.tensor_tensor(td[:], x1, sint, ALU.mult)
                nc.vector.tensor_tensor(dst[:, :, 0:64], ta[:], tb[:],
                                        ALU.subtract)
                nc.vector.tensor_tensor(dst[:, :, 64:128], tc2[:], td[:],
                                        ALU.add)

            k_own = ptmp.tile([128, NKV, 128], BF16, name="k_own",
                              tag="kown", bufs=1)
            rope_tok(pskv[:, 0:512].rearrange("p (h d) -> p h d", h=NKV),
                     kcosR, ksinR, k_own, NKV)
            v_own = ptmp.tile([128, 512], BF16, name="v_own", tag="vb",
                              bufs=1)
            nc.scalar.activation(v_own[:], pskv[:, 512:1024], ACTF.Copy,
                                 scale=rstd0[:])
            kT_own = ptmp.tile([128, NKV, 128], BF16, name="kT_own",
                               tag="cp", bufs=1)
            tpk = psB.tile([128, 4, 128], BF16, name="tpk", tag="B")
            for j in range(NKV):
                nc.tensor.transpose(tpk[:, j, :], k_own[:, j, :], ident_b[:])
            nc.vector.tensor_copy(kT_own[:], tpk[:])
            nc.gpsimd.dma_start(agkv_in[:, 0:512],
                                kT_own[:].rearrange("p a b -> p (a b)"))
            nc.gpsimd.dma_start(agkv_in[:, 512:1024], v_own[:])
            nc.gpsimd.collective_compute(
                "AllGather", ALU.bypass, replica_groups=rg,
                ins=[agkv_in.opt()], outs=[agkv_out.opt()])

            # ===== P1b: q projection (raw) + rstd-folded rope + qT =====
            q_own = ptmp.tile([128, NH, 128], BF16, name="q_own", tag="x",
                              bufs=2)
            for pg in range(2):
                psq = psA.tile([128, 1024], F32, name=f"psq_{pg}", tag="A")
                for hc in range(HC):
                    wqc = pstream.tile([128, 1024], BF16,
                                       name=f"wq_{pg}_{hc}", tag="wstr",
                                       bufs=4)
                    (nc.sync if hc % 2 == 0 else nc.scalar).dma_start(
                        wqc[:], wq_d[hc, :, pg * 1024:(pg + 1) * 1024])
                    for c2 in range(2):
                        nc.tensor.matmul(
                            psq[:, c2 * 512:(c2 + 1) * 512],
                            xT_raw[:, hc, :],
                            wqc[:, c2 * 512:(c2 + 1) * 512],
                            start=(hc == 0), stop=(hc == HC - 1))
                rope_tok(psq[:].rearrange("p (h d) -> p h d", h=8),
                         qcosR, qsinR,
                         q_own[:, pg * 8:(pg + 1) * 8, :], 8)
            qT = ptmp.tile([128, NH, 128], BF16, name="qT", tag="qT", bufs=1)
            for hg in range(4):
                tpq = psB.tile([128, 4, 128], BF16, name=f"tpq_{hg}",
                               tag="B")
                for j in range(4):
                    nc.tensor.transpose(tpq[:, j, :], q_own[:, hg * 4 + j, :],
                                        ident_b[:])
                nc.vector.tensor_copy(qT[:, hg * 4:(hg + 1) * 4, :], tpq[:])

            # ===== P1c: gather k/v of all tokens (k first) =====
            kT_full = pbig.tile([128, NKV, TB, 128], BF16, name="kT_full",
                                tag="kT")
            v_sb = pbig.tile([128, TB, NKV, 128], BF16, name="v_sb",
                             tag="vsb")
            for c in range(N_CORES):
                (nc.sync if c % 2 == 0 else nc.scalar).dma_start(
                    kT_full[:, :, c, :],
                    agkv_out[c, :, 0:512].rearrange("p (a b) -> p a b",
                                                    a=NKV))
            for c in range(N_CORES):
                (nc.sync if c % 2 == 0 else nc.scalar).dma_start(
                    v_sb[:, c, :, :],
                    agkv_out[c, :, 512:1024].rearrange("p (a b) -> p a b",
                                                       a=NKV))

            # prefetch w_dense + shared-expert weights (queues idle now)
            wd_sb = pbig.tile([128, 8, H], BF16, name="wd_sb", tag="xT")
            for ch in range(8):
                (nc.sync if ch % 2 == 0 else nc.scalar).dma_start(
                    wd_sb[:, ch, :], wd_d[ch])
            sw13_sb = pw.tile([128, HC, 256], BF16, name="sw13_sb", tag="wA")
            nc.sync.dma_start(sw13_sb[:], sw13_d[:])
            sw2_sb = pw.tile([128, H], BF16, name="sw2_sb")
            nc.scalar.dma_start(sw2_sb[:], sw2_d[:])

            # ===== P2: attention, 16 heads in kv-sharing pairs =====
            ctxga = pbig.tile([128, NH, 128], BF16, name="ctxga",
                              tag="ctxga")

            def head_scores(h):
                sc = psA.tile([128, T], F32, name=f"sc_{h}", tag="A")
                for c2 in range(2):
                    nc.tensor.matmul(
                        sc[:, c2 * 512:(c2 + 1) * 512], qT[:, h, :],
                        kT_full[:, h // 4, c2 * 4:(c2 + 1) * 4, :],
                        start=True, stop=True)
                sb = ptmp.tile([128, T], BF16, name=f"sb_{h}", tag="wds",
                               bufs=2)
                nc.vector.tensor_tensor(sb[:], sc[:], maskb[:], ALU.add)
                pb = ptmp.tile([128, T], BF16, name=f"pb_{h}", tag="pb",
                               bufs=4)
                rs_ = ptmp.tile([128, 1], F32, name="rs", tag="negm", bufs=8)
                nc.scalar.activation(pb[:], sb[:], ACTF.Exp,
                                     accum_out=rs_[:])
                rinv = ptmp.tile([128, 1], F32, name="rinv", tag="negm",
                                 bufs=8)
                nc.vector.reciprocal(rinv[:], rs_[:])
                nc.vector.tensor_scalar_mul(pb[:], pb[:], rinv[:])
                return pb

            def pair_ctx(p, pb0, pb1):
                """Context for heads 2p, 2p+1 (share kv head p//2)."""
                pT = ptmp.tile([128, TB, 2, 128], BF16, name=f"pT_{p}",
                               tag=f"probsT{p % 2}", bufs=1)
                for hi, pb in ((0, pb0), (1, pb1)):
                    for g4 in range(2):
                        tpp = psB.tile([128, 4, 128], BF16,
                                       name=f"tpp_{p}_{hi}_{g4}", tag="B")
                        for j in range(4):
                            b = g4 * 4 + j
                            nc.tensor.transpose(
                                tpp[:, j, :],
                                pb[:, b * 128:(b + 1) * 128],
                                ident_b[:])
                        eng = nc.vector if (hi + g4) % 2 == 0 else None
                        if eng is not None:
                            eng.tensor_copy(
                                pT[:, g4 * 4:(g4 + 1) * 4, hi, :], tpp[:])
                        else:
                            nc.scalar.activation(
                                pT[:, g4 * 4:(g4 + 1) * 4, hi, :], tpp[:],
                                ACTF.Copy)
                cps = psB.tile([128, 2, 128], F32, name=f"cx_{p}", tag="B")
                for b in range(TB):
                    nc.tensor.matmul(cps[:], v_sb[:, b, p // 2, :],
                                     pT[:, b, :, :],
                                     start=(b == 0), stop=(b == TB - 1))
                if p % 2 == 0:
                    nc.vector.tensor_copy(ctxga[:, 2 * p:2 * p + 2, :],
                                          cps[:])
                else:
                    nc.scalar.activation(ctxga[:, 2 * p:2 * p + 2, :],
                                         cps[:], ACTF.Copy)

            pbs = []
            for h in range(NH):
                pbs.append(head_scores(h))
                if h % 2 == 1 and h >= 3:
                    p = (h - 3) // 2
                    pair_ctx(p, pbs[2 * p], pbs[2 * p + 1])
            pair_ctx(7, pbs[14], pbs[15])

            if debug:
                nc.gpsimd.dma_start(dbg["q"][:], qT[:, 0, :])
            # ============ P3: local dense projection ============
            dh = [psA.tile([128, 1024], F32, name=f"dh_{i}", tag="A")
                  for i in range(2)]
            for ch in range(HC):
                if ch < 8:
                    wh = [wd_sb[:, ch, 0:1024], wd_sb[:, ch, 1024:2048]]
                else:
                    wh = []
                    for half in range(2):
                        wt = pstream.tile([128, 1024], BF16,
                                          name=f"wdc_{ch}_{half}",
                                          tag="wstr", bufs=4)
                        (nc.sync if half == 0 else nc.scalar).dma_start(
                            wt[:], wd_d[ch, :, half * 1024:(half + 1) * 1024])
                        wh.append(wt[:])
                for half in range(2):
                    for c2 in range(2):
                        nc.tensor.matmul(
                            dh[half][:, c2 * 512:(c2 + 1) * 512],
                            ctxga[:, ch, :],
                            wh[half][:, c2 * 512:(c2 + 1) * 512],
                            start=(ch == 0), stop=(ch == HC - 1))

            # ============ P4: resid2, rmsnorm2, router, chunked AG ========
            resid2 = pstream.tile([128, H], F32, name="resid2", tag="hr",
                                  bufs=1)
            nc.vector.tensor_tensor(resid2[:, 0:1024], dh[0][:],
                                    resid_own[:, 0:1024], ALU.add)
            nc.vector.tensor_tensor(resid2[:, 1024:2048], dh[1][:],
                                    resid_own[:, 1024:2048], ALU.add)
            if debug:
                att_f = ptmp.tile([128, H], F32, name="att_f", tag="attdbg",
                                  bufs=1)
                nc.vector.tensor_tensor(att_f[:], resid2[:], resid_own[:],
                                        ALU.subtract)
                nc.gpsimd.dma_start(dbg["att"][:], att_f[:])

            ssum4 = ptmp.tile([128, 1], F32, name="ssum4", tag="nrm", bufs=4)
            sq4 = ptmp.tile([128, H], BF16, name="sq4", tag="x", bufs=2)
            nc.scalar.activation(sq4[:], resid2[:], ACTF.Square,
                                 accum_out=ssum4[:])
            var4 = ptmp.tile([128, 1], F32, name="var4", tag="nrm", bufs=4)
            nc.vector.tensor_scalar(var4[:], ssum4[:], 1.0 / H, RMS_EPS,
                                    ALU.mult, ALU.add)
            sd4 = ptmp.tile([128, 1], F32, name="sd4", tag="nrm", bufs=4)
            nc.scalar.activation(sd4[:], var4[:], ACTF.Sqrt)
            rstd4 = ptmp.tile([128, 1], F32, name="rstd4", tag="nrm", bufs=4)
            nc.vector.reciprocal(rstd4[:], sd4[:])

            x2_f = pbig.tile([128, H], F32, name="x2_f", tag="kT")
            nc.scalar.activation(x2_f[:, 0:HCA * 128],
                                 resid2[:, 0:HCA * 128], ACTF.Copy,
                                 scale=rstd4[:])
            nc.scalar.activation(x2_f[:, HCA * 128:H],
                                 resid2[:, HCA * 128:H], ACTF.Copy,
                                 scale=rstd4[:])
            if debug:
                nc.gpsimd.dma_start(dbg["x2"][:], x2_f[:])
            x2Tf = pbig.tile([128, HC, 128], F32, name="x2Tf", tag="x2Tf")
            x2T_own = ptmp.tile([128, HC, 128], BF16, name="x2T_own",
                                tag="xTown", bufs=1)
            for hg in range(8):
                tp = psB.tile([128, 2, 128], F32, name=f"tpn_{hg}", tag="B")
                for j in range(2):
                    hcc = hg * 2 + j
                    nc.tensor.transpose(
                        tp[:, j, :], x2_f[:, hcc * 128:(hcc + 1) * 128],
                        ident_f[:])
                nc.vector.tensor_copy(x2Tf[:, hg * 2:(hg + 1) * 2, :],
                                      tp[:])
                nc.scalar.activation(x2T_own[:, hg * 2:(hg + 1) * 2, :],
                                     tp[:], ACTF.Copy)
            nc.gpsimd.dma_start(
                ag1_in[:, 0:HC * 128],
                x2T_own[:].rearrange("p a b -> p (a b)"))
            nc.gpsimd.dma_start(out1_d[:], resid2[:])

            # router on own block
            lg = psB.tile([128, E], F32, name="lg", tag="B")
            for hc in range(HC):
                nc.tensor.matmul(lg[:], x2Tf[:, hc, :], gate_sb[:, hc, :],
                                 start=(hc == 0), stop=(hc == HC - 1))
            negm1 = ptmp.tile([128, 1], F32, name="negm1", tag="negm",
                              bufs=8)
            nc.vector.reduce_max(negm1[:], lg[:], axis=X, negate=True)
            ee = ptmp.tile([128, E], F32, name="ee", tag="t_ee", bufs=2)
            nc.scalar.activation(ee[:], lg[:], ACTF.Exp, bias=negm1[:])
            work = ptmp.tile([128, E], F32, name="work", tag="t_wk", bufs=2)
            nc.vector.tensor_copy(work[:], ee[:])
            mth = ptmp.tile([128, 1], F32, name="mth", tag="negm", bufs=8)
            nc.vector.reduce_max(mth[:], work[:], axis=X)
            msk = ptmp.tile([128, E], F32, name="msk", tag="t_mk", bufs=2)
            for _ in range(TOPK - 1):
                nc.vector.tensor_scalar(msk[:], work[:], mth[:], 1e30,
                                        ALU.is_ge, ALU.mult)
                nc.vector.tensor_tensor(work[:], work[:], msk[:],
                                        ALU.subtract)
                nc.vector.reduce_max(mth[:], work[:], axis=X)
            ge = ptmp.tile([128, E], F32, name="ge", tag="t_ge", bufs=2)
            nc.vector.tensor_scalar(ge[:], ee[:], mth[:], None, ALU.is_ge)
            cu = ptmp.tile([128, E], F32, name="cu", tag="t_cu", bufs=2)
            nc.vector.tensor_tensor(cu[:], ee[:], ge[:], ALU.mult)
            s4 = ptmp.tile([128, 1], F32, name="s4", tag="negm", bufs=8)
            nc.vector.reduce_sum(s4[:], cu[:], axis=X)
            ri4 = ptmp.tile([128, 1], F32, name="ri4", tag="negm", bufs=8)
            nc.vector.reciprocal(ri4[:], s4[:])
            combo = ptmp.tile([128, E], BF16, name="combo", tag="t_cb",
                              bufs=2)
            nc.vector.tensor_scalar_mul(combo[:], cu[:], ri4[:])
            nc.gpsimd.dma_start(ag1_in[:, HC * 128:HC * 128 + E],
                                combo[:])
            nc.gpsimd.collective_compute(
                "AllGather", ALU.bypass, replica_groups=rg,
                ins=[ag1_in.opt()], outs=[ag1_out.opt()])

            # ============ P5: unpack x2T + combT from chunked AGs ========
            x2T = pbig.tile([128, HC, TB, 128], BF16, name="x2T", tag="xT")
            for tb2 in range(TB):
                nc.sync.dma_start(
                    x2T[:, 0:HCA, tb2, :],
                    ag1a_out[tb2].rearrange("p (a b) -> p a b", a=HCA))
            cp_sb = ptmp.tile([128, TB, E], BF16, name="cp_sb", tag="cp",
                              bufs=1)
            nc.scalar.dma_start(
                cp_sb[:],
                ag1b_out[:, :, HCB * 128:HCB * 128 + E].rearrange(
                    "a p e -> p a e"))
            for tb2 in range(TB):
                nc.scalar.dma_start(
                    x2T[:, HCA:HC, tb2, :],
                    ag1b_out[tb2, :, 0:HCB * 128].rearrange(
                        "p (a b) -> p a b", a=HCB))
            combT = ptmp.tile([E, TB, 128], F32, name="combT", tag="qT",
                              bufs=1)
            for tb2 in range(TB):
                tpc = psB.tile([E, 128], BF16, name=f"tpc_{tb2}", tag="B")
                nc.tensor.transpose(tpc[:], cp_sb[:, tb2, :], ident_b[:])
                nc.vector.tensor_copy(combT[:, tb2, :], tpc[:])

            # ============ P6: experts + shared ============
            acts = []
            for ei, (w13_d_, sel_sb) in enumerate(
                    ((w13a_d, sela_sb), (w13b_d, selb_sb))):
                bps = psA.tile([128, T], F32, name=f"bps_{ei}", tag="A")
                for half in range(2):
                    nc.tensor.matmul(
                        bps[:, half * 512:(half + 1) * 512],
                        sel_sb[:], combT[:, half * 4:(half + 1) * 4, :],
                        start=True, stop=True)
                cb = ptmp.tile([128, T], F32, name=f"cb_{ei}", tag="cb",
                               bufs=1)
                nc.vector.tensor_copy(cb[:], bps[:])

                act_e = pbig.tile([128, 4, TB, 128], BF16, name=f"act_{ei}",
                                  tag=("x2Tf" if ei == 0 else "ro"))
                for cc in range(4):
                    wt_g = pstream.tile([128, HC, 128], BF16,
                                        name=f"wg_{ei}_{cc}", tag="w13",
                                        bufs=4)
                    nc.sync.dma_start(wt_g[:], w13_d_[cc].rearrange(
                        "p (a b) -> p a b", a=HC))
                    wt_u = pstream.tile([128, HC, 128], BF16,
                                        name=f"wu_{ei}_{cc}", tag="w13",
                                        bufs=4)
                    nc.scalar.dma_start(wt_u[:], w13_d_[cc + 4].rearrange(
                        "p (a b) -> p a b", a=HC))
                    gps = psA.tile([128, T], F32, name=f"g_{ei}_{cc}",
                                   tag="A")
                    ups = psA.tile([128, T], F32, name=f"u_{ei}_{cc}",
                                   tag="A")
                    for hc in range(HC):
                        for half in range(2):
                            nc.tensor.matmul(
                                gps[:, half * 512:(half + 1) * 512],
                                wt_g[:, hc, :],
                                x2T[:, hc, half * 4:(half + 1) * 4, :],
                                start=(hc == 0), stop=(hc == HC - 1))
                    for hc in range(HC):
                        for half in range(2):
                            nc.tensor.matmul(
                                ups[:, half * 512:(half + 1) * 512],
                                wt_u[:, hc, :],
                                x2T[:, hc, half * 4:(half + 1) * 4, :],
                                start=(hc == 0), stop=(hc == HC - 1))
                    sil = ptmp.tile([128, T], F32, name=f"sil_{ei}_{cc}",
                                    tag="x", bufs=2)
                    nc.scalar.activation(sil[:], gps[:], ACTF.Silu)
                    ut = ptmp.tile([128, T], F32, name=f"ut_{ei}_{cc}",
                                   tag="xTown", bufs=1)
                    nc.vector.tensor_tensor(ut[:], ups[:], cb[:], ALU.mult)
                    nc.gpsimd.tensor_tensor(
                        act_e[:, cc, :, :].rearrange("p a b -> p (a b)"),
                        sil[:], ut[:], ALU.mult)
                acts.append(act_e)

            # shared expert (column-parallel)
            act_sh = pbig.tile([128, TB, 128], BF16, name="act_sh",
                               tag="act_sh")
            gps_s = psA.tile([128, T], F32, name="gps_s", tag="A")
            ups_s = psA.tile([128, T], F32, name="ups_s", tag="A")
            for hc in range(HC):
                for col, ps in ((0, gps_s), (1, ups_s)):
                    for half in range(2):
                        nc.tensor.matmul(
                            ps[:, half * 512:(half + 1) * 512],
                            sw13_sb[:, hc, col * 128:(col + 1) * 128],
                            x2T[:, hc, half * 4:(half + 1) * 4, :],
                            start=(hc == 0), stop=(hc == HC - 1))
            sil_s = ptmp.tile([128, T], F32, name="sil_s", tag="x", bufs=2)
            nc.scalar.activation(sil_s[:], gps_s[:], ACTF.Silu)
            nc.vector.tensor_tensor(
                act_sh[:].rearrange("p a b -> p (a b)"), sil_s[:], ups_s[:],
                ALU.mult)

            if debug:
                nc.gpsimd.dma_start(
                    dbg["acta"][:],
                    acts[0][:].rearrange("p a b c -> p (a b c)"))

            # w2 stage: token-major output; 5 uneven column groups
            for g in range(len(W2G)):
                gw, go = W2G[g], W2OFF[g]
                w2g = []
                for ei, w2_d_ in enumerate((w2a_d, w2b_d)):
                    wt = pstream.tile([128, 4, gw], BF16,
                                      name=f"w2_{ei}_{g}", tag=f"w2g{ei}",
                                      bufs=2)
                    (nc.sync if ei == 0 else nc.scalar).dma_start(
                        wt[:], w2_d_[:, :, go:go + gw])
                    w2g.append(wt)
                for tb2 in range(TB):
                    ops = psA.tile([128, 512], F32, name=f"o_{g}_{tb2}",
                                   tag="A")
                    k = 0
                    for ei in range(2):
                        for ic in range(4):
                            nc.tensor.matmul(ops[:, 0:gw],
                                             acts[ei][:, ic, tb2, :],
                                             w2g[ei][:, ic, :],
                                             start=(k == 0), stop=False)
                            k += 1
                    nc.tensor.matmul(ops[:, 0:gw], act_sh[:, tb2, :],
                                     sw2_sb[:, go:go + gw],
                                     start=False, stop=True)
                    oo = ptmp.tile([128, 512], RS2_DT, name=f"oo_{g}_{tb2}",
                                   tag="dout", bufs=3)
                    if tb2 % 2 == 0:
                        nc.vector.tensor_copy(oo[:, 0:gw], ops[:, 0:gw])
                    else:
                        nc.scalar.activation(oo[:, 0:gw], ops[:, 0:gw],
                                             ACTF.Copy)
                    nc.gpsimd.dma_start(
                        rs2_in[g][tb2 * 128:(tb2 + 1) * 128, :],
                        oo[:, 0:gw])
                nc.gpsimd.collective_compute(
                    "AllToAll", ALU.bypass, replica_groups=rg,
                    ins=[rs2_in[g].opt()], outs=[a2a2_out[g].opt()])

            # local 8-way adds per column group, write f32 output directly
            for g in range(len(W2G)):
                gw, go = W2G[g], W2OFF[g]
                acc = psA.tile([128, 512], F32, name=f"acc_{g}", tag="A")
                for hf in range(2):
                    pg = ptmp.tile([128, 4, 512], RS2_DT,
                                   name=f"opart_{g}_{hf}", tag="pgh",
                                   bufs=2)
                    nc.sync.dma_start(
                        pg[:, :, 0:gw],
                        a2a2_out[g][hf * 4:(hf + 1) * 4].rearrange(
                            "a p b -> p a b"))
                    for i in range(4):
                        nc.tensor.matmul(acc[:, 0:gw], ident_b[:],
                                         pg[:, i, 0:gw],
                                         start=(hf == 0 and i == 0),
                                         stop=(hf == 1 and i == 3))
                og = ptmp.tile([128, 512], F32, name=f"og_{g}", tag="rope1",
                               bufs=2)
                nc.vector.tensor_copy(og[:, 0:gw], acc[:, 0:gw])
                nc.gpsimd.dma_start(out0_d[:, go:go + gw], og[:, 0:gw])

    nc.compile()
    return nc


def prep_in_maps(inputs):
    """Shard/marshal full inputs into 8 per-core input maps."""
    f32 = np.float32
    hid = np.asarray(inputs["hidden_states"], f32)
    res = np.asarray(inputs["residual"], f32)
    rms1 = np.asarray(inputs["rms1_w"], f32)
    rms2 = np.asarray(inputs["rms2_w"], f32)
    w_qkv = np.asarray(inputs["w_qkv"], f32) * rms1[:, None]
    w_dense = np.asarray(inputs["w_dense"], f32)
    gate_w = np.asarray(inputs["gate_w"], f32) * rms2[:, None]
    w13 = np.asarray(inputs["w13"], f32) * rms2[None, :, None]
    w2 = np.asarray(inputs["w2"], f32)
    sw13 = np.asarray(inputs["sw13"], f32) * rms2[:, None]
    sw2 = np.asarray(inputs["sw2"], f32)
    pos = np.asarray(inputs["position_ids"]).astype(f32)

    inv_freq = (1.0 / (ROPE_THETA **
                       (np.arange(0, D, 2, dtype=f32) / D))).astype(f32)
    ang = pos[:, None] * inv_freq[None, :]          # [T, 64]
    cosa = np.cos(ang).astype(f32)
    sina = np.sin(ang).astype(f32)
    s = np.float32(D ** -0.5)

    gate_dev = np.ascontiguousarray(
        gate_w.reshape(HC, 128, E).transpose(1, 0, 2))

    def bf(x):
        return np.ascontiguousarray(x.astype(BF16_NP))

    wkv_dev = bf(w_qkv[:, Q_SIZE:].reshape(HC, 128, 1024))
    wq_dev = bf(w_qkv[:, :Q_SIZE].reshape(HC, 128, Q_SIZE))
    wd_dev = bf(w_dense.reshape(HC, 128, H))

    in_maps = []
    for c in range(N_CORES):
        tok = slice(128 * c, 128 * c + 128)
        co = np.tile(cosa[tok], (1, 8)).reshape(128, 8, 64)
        si = np.tile(sina[tok], (1, 8)).reshape(128, 8, 64)
        causal = (np.arange(T)[None, :]
                  <= (128 * c + np.arange(128))[:, None])
        maskb = np.where(causal, -4.0, -60.0).astype(f32)

        sw13_c = np.concatenate(
            [sw13[:, 128 * c:128 * c + 128],
             sw13[:, SI + 128 * c:SI + 128 * c + 128]], 1)  # [2048, 256]
        sw13_dev = bf(sw13_c.reshape(HC, 128, 256).transpose(1, 0, 2))
        sw2_dev = bf(sw2[128 * c:128 * c + 128, :])       # [128, 2048]

        def w13_dev(e):
            m = w13[e]                                    # [2048, 1024]
            return bf(m.reshape(HC, 128, 8, 128).transpose(2, 1, 0, 3)
                      .reshape(8, 128, H))

        def w2_dev(e):
            m = w2[e]                                     # [512, 2048]
            return bf(m.reshape(4, 128, H).transpose(1, 0, 2))

        sel = np.zeros((2, E, 128), f32)
        sel[0, 2 * c, :] = 1.0
        sel[1, 2 * c + 1, :] = 1.0

        in_maps.append({
            "resid_own": np.ascontiguousarray(hid[tok] + res[tok]),
            "wkv": wkv_dev, "wq": wq_dev, "wdense": wd_dev,
            "sw13": sw13_dev, "sw2": sw2_dev,
            "w13a": w13_dev(2 * c), "w13b": w13_dev(2 * c + 1),
            "w2a": w2_dev(2 * c), "w2b": w2_dev(2 * c + 1),
            "gate": gate_dev,
            "qcos": np.ascontiguousarray(co * s),
            "qsin": np.ascontiguousarray(si * s),
            "kcos": np.ascontiguousarray(co[:, 0:4, :]),
            "ksin": np.ascontiguousarray(si[:, 0:4, :]),
            "maskb": bf(maskb),
            "sela": np.ascontiguousarray(sel[0]),
            "selb": np.ascontiguousarray(sel[1]),
        })
    return in_maps


_NC_CACHE = {}


def _get_nc(debug=False):
    key = debug
    if key not in _NC_CACHE:
        _NC_CACHE[key] = build_nc(debug=debug)
    return _NC_CACHE[key]


def run(inputs, debug=False, trace=False):
    nc = _get_nc(debug=debug)
    in_maps = prep_in_maps(inputs)
    kw = {}
    if trace:
        kw["trace"] = True
    res = run_bass_kernel_spmd(nc, in_maps, core_ids=list(range(N_CORES)),
                               **kw)
    out0 = np.concatenate([res.results[c]["out0"] for c in range(N_CORES)], 0)
    out1 = np.concatenate([res.results[c]["out1"] for c in range(N_CORES)], 0)
    return (out0, out1), res


def kernel(**inputs):
    (out0, out1), _ = run(inputs)
    return out0, out1


# revision 31
# speedup vs baseline: 1.0158x; 1.0158x over previous
"""Trainium2 Bass kernel for nn_BailingMoeBlock (8 NeuronCores).

Sharding (v5):
  - rmsnorm1: transposes run on RAW resid (bf16); 1/rms folded into the
    rope cos/sin tables and the v copy, so the PE never waits on the
    variance reduction.
  - Attention: token-parallel (own 128 q x all 1024 kv); single kv
    AllGather.  Softmax: additive causal mask (-4 exp-bias folded in, -60
    masked), exp with accum_out row-sums, DVE renormalize.  Heads run in
    kv-sharing pairs; pair context = one N=256 matmul per kv block;
    depth-2 head pipeline.  All staging DMAs stay on the sync/scalar
    hardware-DGE queues (gpsimd SWDGE staging faults on this runtime).
  - resid2 / rmsnorm2 / router on the owned block; x2^T + router weights
    leave in two chunked AllGathers (hc 0..9, then hc 10..15 + comb).
  - MoE: expert-parallel (2 experts/core), hc-outer/half-inner loops so
    each LDWEIGHTS serves two N=512 matmuls; shared expert
    column-parallel.  w2 emits 5 uneven column groups
    (512,512,512,384,128) -> 5 AllToAlls; the tiny last group shrinks the
    exposed tail.  Local 8-way adds via identity matmuls.

Heavy matmuls in bf16 (fp32 accumulate in PSUM); router in fp32.
"""
import sys
import numpy as np

for _p in ("/opt/trn_rl_repo", "/opt/pypackages"):
    if _p not in sys.path:
        sys.path.append(_p)

import ml_dtypes  # noqa: E402
from concourse import bacc  # noqa: E402
import concourse.bass as bass  # noqa: E402
import concourse.tile as tile  # noqa: E402
import concourse.mybir as mybir  # noqa: E402
from concourse.bass_utils import run_bass_kernel_spmd  # noqa: E402
from concourse.masks import make_identity  # noqa: E402

F32 = mybir.dt.float32
BF16 = mybir.dt.bfloat16
BF16_NP = ml_dtypes.bfloat16

N_CORES = 8
T = 1024
H = 2048
NH = 16
NKV = 4
D = 128
E = 16
I_EXP = 512
SI = 1024
TOPK = 4
RMS_EPS = 1e-6
ROPE_THETA = 10000.0
Q_SIZE = NH * D          # 2048
KV_SIZE = NKV * D        # 512
TB = T // 128            # 8 token blocks
HC = H // 128            # 16 h chunks
NEG_BIG = -30000.0
HCA = 10                 # x2T chunks in first AG1 chunk
HCB = HC - HCA           # remaining chunks (ride with router weights)
W2G = (512, 512, 512, 384, 128)   # w2 column-group widths
W2OFF = (0, 512, 1024, 1536, 1920)

RS2_DT = BF16

X = mybir.AxisListType.X
ALU = mybir.AluOpType
ACTF = mybir.ActivationFunctionType


def build_nc(debug=False):
    nc = bacc.Bacc("TRN2", target_bir_lowering=False, debug=False,
                   num_devices=N_CORES)

    def din(name, shape, dt):
        return nc.dram_tensor(name, list(shape), dt, kind="ExternalInput").ap()

    def dout(name, shape, dt):
        return nc.dram_tensor(name, list(shape), dt, kind="ExternalOutput").ap()

    wkv_d = din("wkv", (HC, 128, 1024), BF16)    # k|v columns of w_qkv
    wq_d = din("wq", (HC, 128, 2048), BF16)      # q columns of w_qkv
    wd_d = din("wdense", (HC, 128, H), BF16)     # full w_dense, chunk-major
    sw13_d = din("sw13", (128, HC, 256), BF16)
    sw2_d = din("sw2", (128, H), BF16)
    w13a_d = din("w13a", (8, 128, H), BF16)   # [cc][p][hc*128+ci]
    w13b_d = din("w13b", (8, 128, H), BF16)
    w2a_d = din("w2a", (128, 4, H), BF16)
    w2b_d = din("w2b", (128, 4, H), BF16)
    gate_d = din("gate", (128, HC, E), F32)
    resid_own_d = din("resid_own", (128, H), F32)  # host hid+res own block
    qcos_d = din("qcos", (128, 8, 64), F32)   # own-token cos*scale, x8 heads
    qsin_d = din("qsin", (128, 8, 64), F32)
    kcos_d = din("kcos", (128, 4, 64), F32)
    ksin_d = din("ksin", (128, 4, 64), F32)
    maskb_d = din("maskb", (128, T), BF16)    # additive mask: -4 / -30004
    sela_d = din("sela", (E, 128), F32)
    selb_d = din("selb", (E, 128), F32)

    out0_d = dout("out0", (128, H), F32)
    out1_d = dout("out1", (128, H), F32)
    dbg = {}
    if debug:
        dbg["q"] = dout("dbg_q", (128, 128), F32)        # qT head0 (own toks)
        dbg["att"] = dout("dbg_att", (128, H), F32)      # attn_out own block
        dbg["x2"] = dout("dbg_x2", (128, H), F32)        # x2 own block fp32
        dbg["comb"] = dout("dbg_comb", (128, E), F32)    # comb own block
        dbg["acta"] = dout("dbg_acta", (128, 4 * T), F32)  # act expert a

    with tile.TileContext(nc) as tc:
        with (
            tc.tile_pool(name="const", bufs=1) as pc,
            tc.tile_pool(name="weights", bufs=1) as pw,
            tc.tile_pool(name="big", bufs=1) as pbig,
            tc.tile_pool(name="stream", bufs=3) as pstream,
            tc.tile_pool(name="tmp", bufs=2) as ptmp,
            tc.tile_pool(name="psA", bufs=3, space="PSUM") as psA,
            tc.tile_pool(name="psB", bufs=2, space="PSUM") as psB,
            tc.tile_pool(name="dram", bufs=1, space="DRAM") as pd,
        ):
            # ---------------- constants (scalar/ACT DMA queue) -------------
            ident_b = pc.tile([128, 128], BF16, name="ident_b")
            make_identity(nc, ident_b[:])
            ident_f = pc.tile([128, 128], F32, name="ident_f")
            make_identity(nc, ident_f[:])
            qcos = pc.tile([128, 8, 64], F32, name="qcos")
            nc.scalar.dma_start(qcos[:], qcos_d[:])
            qsin = pc.tile([128, 8, 64], F32, name="qsin")
            nc.scalar.dma_start(qsin[:], qsin_d[:])
            kcos = pc.tile([128, 4, 64], F32, name="kcos")
            nc.scalar.dma_start(kcos[:], kcos_d[:])
            ksin = pc.tile([128, 4, 64], F32, name="ksin")
            nc.scalar.dma_start(ksin[:], ksin_d[:])
            maskb = pc.tile([128, T], BF16, name="maskb")
            nc.scalar.dma_start(maskb[:], maskb_d[:])
            gate_sb = pc.tile([128, HC, E], F32, name="gate_sb")
            nc.scalar.dma_start(gate_sb[:], gate_d[:])
            sela_sb = pc.tile([E, 128], F32, name="sela_sb")
            nc.scalar.dma_start(sela_sb[:], sela_d[:])
            selb_sb = pc.tile([E, 128], F32, name="selb_sb")
            nc.scalar.dma_start(selb_sb[:], selb_d[:])

            # ---------------- DRAM internal buffers ----------------
            agkv_in = pd.tile([128, 1024], BF16, name="agkv_in")
            agkv_out = pd.tile([N_CORES, 128, 1024], BF16, name="agkv_out",
                               addr_space="Shared")
            ag1a_in = pd.tile([128, HCA * 128], BF16, name="ag1a_in")
            ag1a_out = pd.tile([TB, 128, HCA * 128], BF16,
                               name="ag1a_out", addr_space="Shared")
            ag1b_in = pd.tile([128, HCB * 128 + E], BF16, name="ag1b_in")
            ag1b_out = pd.tile([TB, 128, HCB * 128 + E], BF16,
                               name="ag1b_out", addr_space="Shared")
            A2AG = ((0, 1024), (1024, 512), (1536, 512))
            rs2_in = [pd.tile([T, gw], RS2_DT, name=f"rs2_in{g}")
                      for g, (go, gw) in enumerate(A2AG)]
            a2a2_out = [pd.tile([TB, 128, gw], RS2_DT,
                                name=f"a2a2_out{g}")
                        for g, (go, gw) in enumerate(A2AG)]
            rg = [list(range(N_CORES))]

            # ===== P0: own-block resid; transposes on RAW resid =====
            resid_own = pbig.tile([128, H], F32, name="resid_own", tag="ro")
            nc.sync.dma_start(resid_own[:, 0:1024], resid_own_d[:, 0:1024])
            nc.sync.dma_start(resid_own[:, 1024:2048],
                              resid_own_d[:, 1024:2048])
            rb = ptmp.tile([128, H], BF16, name="rb", tag="x", bufs=2)
            nc.scalar.activation(rb[:, 0:1024], resid_own[:, 0:1024],
                                 ACTF.Copy)
            nc.scalar.activation(rb[:, 1024:2048], resid_own[:, 1024:2048],
                                 ACTF.Copy)
            # variance (off the transpose critical path)
            sqj = ptmp.tile([128, H], BF16, name="sqj", tag="x", bufs=2)
            ssum0 = ptmp.tile([128, 1], F32, name="ssum0", tag="nrm", bufs=4)
            nc.scalar.activation(sqj[:], resid_own[:], ACTF.Square,
                                 accum_out=ssum0[:])
            var0 = ptmp.tile([128, 1], F32, name="var0", tag="nrm", bufs=4)
            nc.vector.tensor_scalar(var0[:], ssum0[:], 1.0 / H, RMS_EPS,
                                    ALU.mult, ALU.add)
            sd0 = ptmp.tile([128, 1], F32, name="sd0", tag="nrm", bufs=4)
            nc.scalar.activation(sd0[:], var0[:], ACTF.Sqrt)
            rstd0 = ptmp.tile([128, 1], F32, name="rstd0", tag="nrm", bufs=4)
            nc.vector.reciprocal(rstd0[:], sd0[:])
            # rstd-folded rope tables (one tile; overlays P6's cb slot)
            cosR_all = ptmp.tile([128, 24, 64], F32, name="cosR_all",
                                 tag="cb", bufs=1)
            qcosR = cosR_all[:, 0:8, :]
            qsinR = cosR_all[:, 8:16, :]
            kcosR = cosR_all[:, 16:20, :]
            ksinR = cosR_all[:, 20:24, :]
            nc.vector.tensor_scalar_mul(qcosR, qcos[:], rstd0[:])
            nc.vector.tensor_scalar_mul(qsinR, qsin[:], rstd0[:])
            nc.vector.tensor_scalar_mul(kcosR, kcos[:], rstd0[:])
            nc.vector.tensor_scalar_mul(ksinR, ksin[:], rstd0[:])

            xT_raw = ptmp.tile([128, HC, 128], BF16, name="xT_raw",
                               tag="xTown", bufs=1)
            for hg in range(4):
                tp = psB.tile([128, 4, 128], BF16, name=f"tpx_{hg}", tag="B")
                for j in range(4):
                    hcc = hg * 4 + j
                    nc.tensor.transpose(
                        tp[:, j, :], rb[:, hcc * 128:(hcc + 1) * 128],
                        ident_b[:])
                if hg % 2 == 0:
                    nc.vector.tensor_copy(xT_raw[:, hg * 4:(hg + 1) * 4, :],
                                          tp[:])
                else:
                    nc.scalar.activation(xT_raw[:, hg * 4:(hg + 1) * 4, :],
                                         tp[:], ACTF.Copy)

            # ===== P1a: kv projection (raw) + rstd-folded rope + AGk/AGv ===
            pskv = psA.tile([128, 1024], F32, name="pskv", tag="A")
            for hc in range(HC):
                wkvc = pstream.tile([128, 1024], BF16, name=f"wkv_{hc}",
                                    tag="wstr", bufs=4)
                (nc.sync if hc % 2 == 0 else nc.scalar).dma_start(
                    wkvc[:], wkv_d[hc])
                for c2 in range(2):
                    nc.tensor.matmul(
                        pskv[:, c2 * 512:(c2 + 1) * 512], xT_raw[:, hc, :],
                        wkvc[:, c2 * 512:(c2 + 1) * 512],
                        start=(hc == 0), stop=(hc == HC - 1))

            def rope_tok(pview, cost, sint, dst, nh):
                """pview [128, nh, 128] psum; dst [128, nh, 128] bf16 sbuf."""
                x1 = pview[:, :, 0:64]
                x2 = pview[:, :, 64:128]
                ta = ptmp.tile([128, nh, 64], F32, name="ta", tag="rope1",
                               bufs=2)
                tb = ptmp.tile([128, nh, 64], F32, name="tb", tag="rope2",
                               bufs=2)
                tc2 = ptmp.tile([128, nh, 64], F32, name="tc", tag="rope1",
                                bufs=2)
                td = ptmp.tile([128, nh, 64], F32, name="td", tag="rope2",
                               bufs=2)
                nc.vector.tensor_tensor(ta[:], x1, cost, ALU.mult)
                nc.vector.tensor_tensor(tb[:], x2, sint, ALU.mult)
                nc.vector.tensor_tensor(tc2[:], x2, cost, ALU.mult)
                nc.vector.tensor_tensor(td[:], x1, sint, ALU.mult)
                nc.vector.tensor_tensor(dst[:, :, 0:64], ta[:], tb[:],
                                        ALU.subtract)
                nc.vector.tensor_tensor(dst[:, :, 64:128], tc2[:], td[:],
                                        ALU.add)

            k_own = ptmp.tile([128, NKV, 128], BF16, name="k_own",
                              tag="kown", bufs=1)
            rope_tok(pskv[:, 0:512].rearrange("p (h d) -> p h d", h=NKV),
                     kcosR, ksinR, k_own, NKV)
            v_own = ptmp.tile([128, 512], BF16, name="v_own", tag="vb",
                              bufs=1)
            nc.scalar.activation(v_own[:], pskv[:, 512:1024], ACTF.Copy,
                                 scale=rstd0[:])
            kT_own = ptmp.tile([128, NKV, 128], BF16, name="kT_own",
                               tag="cp", bufs=1)
            tpk = psB.tile([128, 4, 128], BF16, name="tpk", tag="B")
            for j in range(NKV):
                nc.tensor.transpose(tpk[:, j, :], k_own[:, j, :], ident_b[:])
            nc.vector.tensor_copy(kT_own[:], tpk[:])
            nc.gpsimd.dma_start(agkv_in[:, 0:512],
                                kT_own[:].rearrange("p a b -> p (a b)"))
            nc.gpsimd.dma_start(agkv_in[:, 512:1024], v_own[:])
            nc.gpsimd.collective_compute(
                "AllGather", ALU.bypass, replica_groups=rg,
                ins=[agkv_in.opt()], outs=[agkv_out.opt()])

            # ===== P1b: q projection (raw) + rstd-folded rope + qT =====
            q_own = ptmp.tile([128, NH, 128], BF16, name="q_own", tag="x",
                              bufs=2)
            for pg in range(2):
                psq = psA.tile([128, 1024], F32, name=f"psq_{pg}", tag="A")
                for hc in range(HC):
                    wqc = pstream.tile([128, 1024], BF16,
                                       name=f"wq_{pg}_{hc}", tag="wstr",
                                       bufs=4)
                    (nc.sync if hc % 2 == 0 else nc.scalar).dma_start(
                        wqc[:], wq_d[hc, :, pg * 1024:(pg + 1) * 1024])
                    for c2 in range(2):
                        nc.tensor.matmul(
                            psq[:, c2 * 512:(c2 + 1) * 512],
                            xT_raw[:, hc, :],
                            wqc[:, c2 * 512:(c2 + 1) * 512],
                            start=(hc == 0), stop=(hc == HC - 1))
                rope_tok(psq[:].rearrange("p (h d) -> p h d", h=8),
                         qcosR, qsinR,
                         q_own[:, pg * 8:(pg + 1) * 8, :], 8)
            qT = ptmp.tile([128, NH, 128], BF16, name="qT", tag="qT", bufs=1)
            for hg in range(4):
                tpq = psB.tile([128, 4, 128], BF16, name=f"tpq_{hg}",
                               tag="B")
                for j in range(4):
                    nc.tensor.transpose(tpq[:, j, :], q_own[:, hg * 4 + j, :],
                                        ident_b[:])
                nc.vector.tensor_copy(qT[:, hg * 4:(hg + 1) * 4, :], tpq[:])

            # ===== P1c: gather k/v of all tokens (k first) =====
            kT_full = pbig.tile([128, NKV, TB, 128], BF16, name="kT_full",
                                tag="kT")
            v_sb = pbig.tile([128, TB, NKV, 128], BF16, name="v_sb",
                             tag="vsb")
            for c in range(N_CORES):
                (nc.sync if c % 2 == 0 else nc.scalar).dma_start(
                    kT_full[:, :, c, :],
                    agkv_out[c, :, 0:512].rearrange("p (a b) -> p a b",
                                                    a=NKV))
            for c in range(N_CORES):
                (nc.sync if c % 2 == 0 else nc.scalar).dma_start(
                    v_sb[:, c, :, :],
                    agkv_out[c, :, 512:1024].rearrange("p (a b) -> p a b",
                                                       a=NKV))

            # prefetch w_dense + shared-expert weights (queues idle now)
            wd_sb = pbig.tile([128, 8, H], BF16, name="wd_sb", tag="xT")
            for ch in range(8):
                (nc.sync if ch % 2 == 0 else nc.scalar).dma_start(
                    wd_sb[:, ch, :], wd_d[ch])
            sw13_sb = pw.tile([128, HC, 256], BF16, name="sw13_sb", tag="wA")
            nc.sync.dma_start(sw13_sb[:], sw13_d[:])
            sw2_sb = pw.tile([128, H], BF16, name="sw2_sb")
            nc.scalar.dma_start(sw2_sb[:], sw2_d[:])

            # ===== P2: attention, 16 heads in kv-sharing pairs =====
            ctxga = pbig.tile([128, NH, 128], BF16, name="ctxga",
                              tag="ctxga")

            def head_scores(h):
                sc = psA.tile([128, T], F32, name=f"sc_{h}", tag="A")
                for c2 in range(2):
                    nc.tensor.matmul(
                        sc[:, c2 * 512:(c2 + 1) * 512], qT[:, h, :],
                        kT_full[:, h // 4, c2 * 4:(c2 + 1) * 4, :],
                        start=True, stop=True)
                sb = ptmp.tile([128, T], BF16, name=f"sb_{h}", tag="wds",
                               bufs=2)
                nc.vector.tensor_tensor(sb[:], sc[:], maskb[:], ALU.add)
                pb = ptmp.tile([128, T], BF16, name=f"pb_{h}", tag="pb",
                               bufs=4)
                rs_ = ptmp.tile([128, 1], F32, name="rs", tag="negm", bufs=8)
                nc.scalar.activation(pb[:], sb[:], ACTF.Exp,
                                     accum_out=rs_[:])
                rinv = ptmp.tile([128, 1], F32, name="rinv", tag="negm",
                                 bufs=8)
                nc.vector.reciprocal(rinv[:], rs_[:])
                nc.vector.tensor_scalar_mul(pb[:], pb[:], rinv[:])
                return pb

            def pair_ctx(p, pb0, pb1):
                """Context for heads 2p, 2p+1 (share kv head p//2)."""
                pT = ptmp.tile([128, TB, 2, 128], BF16, name=f"pT_{p}",
                               tag=f"probsT{p % 2}", bufs=1)
                for hi, pb in ((0, pb0), (1, pb1)):
                    for g4 in range(2):
                        tpp = psB.tile([128, 4, 128], BF16,
                                       name=f"tpp_{p}_{hi}_{g4}", tag="B")
                        for j in range(4):
                            b = g4 * 4 + j
                            nc.tensor.transpose(
                                tpp[:, j, :],
                                pb[:, b * 128:(b + 1) * 128],
                                ident_b[:])
                        eng = nc.vector if (hi + g4) % 2 == 0 else None
                        if eng is not None:
                            eng.tensor_copy(
                                pT[:, g4 * 4:(g4 + 1) * 4, hi, :], tpp[:])
                        else:
                            nc.scalar.activation(
                                pT[:, g4 * 4:(g4 + 1) * 4, hi, :], tpp[:],
                                ACTF.Copy)
                cps = psB.tile([128, 2, 128], F32, name=f"cx_{p}", tag="B")
                for b in range(TB):
                    nc.tensor.matmul(cps[:], v_sb[:, b, p // 2, :],
                                     pT[:, b, :, :],
                                     start=(b == 0), stop=(b == TB - 1))
                if p % 2 == 0:
                    nc.vector.tensor_copy(ctxga[:, 2 * p:2 * p + 2, :],
                                          cps[:])
                else:
                    nc.scalar.activation(ctxga[:, 2 * p:2 * p + 2, :],
                                         cps[:], ACTF.Copy)

            pbs = []
            for h in range(NH):
                pbs.append(head_scores(h))
                if h % 2 == 1 and h >= 3:
                    p = (h - 3) // 2
                    pair_ctx(p, pbs[2 * p], pbs[2 * p + 1])
            pair_ctx(7, pbs[14], pbs[15])

            if debug:
                nc.gpsimd.dma_start(dbg["q"][:], qT[:, 0, :])
            # ============ P3: local dense projection ============
            dh = [psA.tile([128, 1024], F32, name=f"dh_{i}", tag="A")
                  for i in range(2)]
            for ch in range(HC):
                if ch < 8:
                    wh = [wd_sb[:, ch, 0:1024], wd_sb[:, ch, 1024:2048]]
                else:
                    wh = []
                    for half in range(2):
                        wt = pstream.tile([128, 1024], BF16,
                                          name=f"wdc_{ch}_{half}",
                                          tag="wstr", bufs=4)
                        (nc.sync if half == 0 else nc.scalar).dma_start(
                            wt[:], wd_d[ch, :, half * 1024:(half + 1) * 1024])
                        wh.append(wt[:])
                for half in range(2):
                    for c2 in range(2):
                        nc.tensor.matmul(
                            dh[half][:, c2 * 512:(c2 + 1) * 512],
                            ctxga[:, ch, :],
                            wh[half][:, c2 * 512:(c2 + 1) * 512],
                            start=(ch == 0), stop=(ch == HC - 1))

            # ============ P4: resid2, rmsnorm2, router, chunked AG ========
            resid2 = pstream.tile([128, H], F32, name="resid2", tag="hr",
                                  bufs=1)
            nc.vector.tensor_tensor(resid2[:, 0:1024], dh[0][:],
                                    resid_own[:, 0:1024], ALU.add)
            nc.vector.tensor_tensor(resid2[:, 1024:2048], dh[1][:],
                                    resid_own[:, 1024:2048], ALU.add)
            if debug:
                att_f = ptmp.tile([128, H], F32, name="att_f", tag="attdbg",
                                  bufs=1)
                nc.vector.tensor_tensor(att_f[:], resid2[:], resid_own[:],
                                        ALU.subtract)
                nc.gpsimd.dma_start(dbg["att"][:], att_f[:])

            ssum4 = ptmp.tile([128, 1], F32, name="ssum4", tag="nrm", bufs=4)
            sq4 = ptmp.tile([128, H], BF16, name="sq4", tag="x", bufs=2)
            nc.scalar.activation(sq4[:], resid2[:], ACTF.Square,
                                 accum_out=ssum4[:])
            var4 = ptmp.tile([128, 1], F32, name="var4", tag="nrm", bufs=4)
            nc.vector.tensor_scalar(var4[:], ssum4[:], 1.0 / H, RMS_EPS,
                                    ALU.mult, ALU.add)
            sd4 = ptmp.tile([128, 1], F32, name="sd4", tag="nrm", bufs=4)
            nc.scalar.activation(sd4[:], var4[:], ACTF.Sqrt)
            rstd4 = ptmp.tile([128, 1], F32, name="rstd4", tag="nrm", bufs=4)
            nc.vector.reciprocal(rstd4[:], sd4[:])

            x2_f = pbig.tile([128, H], F32, name="x2_f", tag="kT")
            nc.scalar.activation(x2_f[:, 0:HCA * 128],
                                 resid2[:, 0:HCA * 128], ACTF.Copy,
                                 scale=rstd4[:])
            nc.scalar.activation(x2_f[:, HCA * 128:H],
                                 resid2[:, HCA * 128:H], ACTF.Copy,
                                 scale=rstd4[:])
            if debug:
                nc.gpsimd.dma_start(dbg["x2"][:], x2_f[:])
            x2Tf = pbig.tile([128, HC, 128], F32, name="x2Tf", tag="x2Tf")
            x2T_own = ptmp.tile([128, HC, 128], BF16, name="x2T_own",
                                tag="xTown", bufs=1)
            for hg in range(8):
                tp = psB.tile([128, 2, 128], F32, name=f"tpn_{hg}", tag="B")
                for j in range(2):
                    hcc = hg * 2 + j
                    nc.tensor.transpose(
                        tp[:, j, :], x2_f[:, hcc * 128:(hcc + 1) * 128],
                        ident_f[:])
                nc.vector.tensor_copy(x2Tf[:, hg * 2:(hg + 1) * 2, :],
                                      tp[:])
                nc.scalar.activation(x2T_own[:, hg * 2:(hg + 1) * 2, :],
                                     tp[:], ACTF.Copy)
            nc.gpsimd.dma_start(
                ag1_in[:, 0:HC * 128],
                x2T_own[:].rearrange("p a b -> p (a b)"))
            nc.gpsimd.dma_start(out1_d[:], resid2[:])

            # router on own block
            lg = psB.tile([128, E], F32, name="lg", tag="B")
            for hc in range(HC):
                nc.tensor.matmul(lg[:], x2Tf[:, hc, :], gate_sb[:, hc, :],
                                 start=(hc == 0), stop=(hc == HC - 1))
            negm1 = ptmp.tile([128, 1], F32, name="negm1", tag="negm",
                              bufs=8)
            nc.vector.reduce_max(negm1[:], lg[:], axis=X, negate=True)
            ee = ptmp.tile([128, E], F32, name="ee", tag="t_ee", bufs=2)
            nc.scalar.activation(ee[:], lg[:], ACTF.Exp, bias=negm1[:])
            work = ptmp.tile([128, E], F32, name="work", tag="t_wk", bufs=2)
            nc.vector.tensor_copy(work[:], ee[:])
            mth = ptmp.tile([128, 1], F32, name="mth", tag="negm", bufs=8)
            nc.vector.reduce_max(mth[:], work[:], axis=X)
            msk = ptmp.tile([128, E], F32, name="msk", tag="t_mk", bufs=2)
            for _ in range(TOPK - 1):
                nc.vector.tensor_scalar(msk[:], work[:], mth[:], 1e30,
                                        ALU.is_ge, ALU.mult)
                nc.vector.tensor_tensor(work[:], work[:], msk[:],
                                        ALU.subtract)
                nc.vector.reduce_max(mth[:], work[:], axis=X)
            ge = ptmp.tile([128, E], F32, name="ge", tag="t_ge", bufs=2)
            nc.vector.tensor_scalar(ge[:], ee[:], mth[:], None, ALU.is_ge)
            cu = ptmp.tile([128, E], F32, name="cu", tag="t_cu", bufs=2)
            nc.vector.tensor_tensor(cu[:], ee[:], ge[:], ALU.mult)
            s4 = ptmp.tile([128, 1], F32, name="s4", tag="negm", bufs=8)
            nc.vector.reduce_sum(s4[:], cu[:], axis=X)
            ri4 = ptmp.tile([128, 1], F32, name="ri4", tag="negm", bufs=8)
            nc.vector.reciprocal(ri4[:], s4[:])
            combo = ptmp.tile([128, E], BF16, name="combo", tag="t_cb",
                              bufs=2)
            nc.vector.tensor_scalar_mul(combo[:], cu[:], ri4[:])
            nc.gpsimd.dma_start(ag1_in[:, HC * 128:HC * 128 + E],
                                combo[:])
            nc.gpsimd.collective_compute(
                "AllGather", ALU.bypass, replica_groups=rg,
                ins=[ag1_in.opt()], outs=[ag1_out.opt()])

            # ============ P5: unpack x2T + combT from chunked AGs ========
            x2T = pbig.tile([128, HC, TB, 128], BF16, name="x2T", tag="xT")
            for tb2 in range(TB):
                nc.sync.dma_start(
                    x2T[:, 0:HCA, tb2, :],
                    ag1a_out[tb2].rearrange("p (a b) -> p a b", a=HCA))
            cp_sb = ptmp.tile([128, TB, E], BF16, name="cp_sb", tag="cp",
                              bufs=1)
            nc.scalar.dma_start(
                cp_sb[:],
                ag1b_out[:, :, HCB * 128:HCB * 128 + E].rearrange(
                    "a p e -> p a e"))
            for tb2 in range(TB):
                nc.scalar.dma_start(
                    x2T[:, HCA:HC, tb2, :],
                    ag1b_out[tb2, :, 0:HCB * 128].rearrange(
                        "p (a b) -> p a b", a=HCB))
            combT = ptmp.tile([E, TB, 128], F32, name="combT", tag="qT",
                              bufs=1)
            for tb2 in range(TB):
                tpc = psB.tile([E, 128], BF16, name=f"tpc_{tb2}", tag="B")
                nc.tensor.transpose(tpc[:], cp_sb[:, tb2, :], ident_b[:])
                nc.vector.tensor_copy(combT[:, tb2, :], tpc[:])

            # ============ P6: experts + shared ============
            acts = []
            for ei, (w13_d_, sel_sb) in enumerate(
                    ((w13a_d, sela_sb), (w13b_d, selb_sb))):
                act_e = pbig.tile([128, 4, TB, 128], BF16, name=f"act_{ei}",
                                  tag=("x2Tf" if ei == 0 else "ro"))
                for cc in range(4):
                    wt_g = pstream.tile([128, HC, 128], BF16,
                                        name=f"wg_{ei}_{cc}", tag="w13",
                                        bufs=4)
                    nc.sync.dma_start(wt_g[:], w13_d_[cc].rearrange(
                        "p (a b) -> p a b", a=HC))
                    wt_u = pstream.tile([128, HC, 128], BF16,
                                        name=f"wu_{ei}_{cc}", tag="w13",
                                        bufs=4)
                    nc.scalar.dma_start(wt_u[:], w13_d_[cc + 4].rearrange(
                        "p (a b) -> p a b", a=HC))
                    gps = psA.tile([128, T], F32, name=f"g_{ei}_{cc}",
                                   tag="A")
                    ups = psA.tile([128, T], F32, name=f"u_{ei}_{cc}",
                                   tag="A")
                    for hc in range(HC):
                        for half in range(2):
                            nc.tensor.matmul(
                                gps[:, half * 512:(half + 1) * 512],
                                wt_g[:, hc, :],
                                x2T[:, hc, half * 4:(half + 1) * 4, :],
                                start=(hc == 0), stop=(hc == HC - 1))
                    if cc == 0:
                        bps = psA.tile([128, T], F32, name=f"bps_{ei}",
                                       tag="A")
                        for half in range(2):
                            nc.tensor.matmul(
                                bps[:, half * 512:(half + 1) * 512],
                                sel_sb[:],
                                combT[:, half * 4:(half + 1) * 4, :],
                                start=True, stop=True)
                        cb = ptmp.tile([128, T], F32, name=f"cb_{ei}",
                                       tag="cb", bufs=1)
                        nc.vector.tensor_copy(cb[:], bps[:])
                    for hc in range(HC):
                        for half in range(2):
                            nc.tensor.matmul(
                                ups[:, half * 512:(half + 1) * 512],
                                wt_u[:, hc, :],
                                x2T[:, hc, half * 4:(half + 1) * 4, :],
                                start=(hc == 0), stop=(hc == HC - 1))
                    sil = ptmp.tile([128, T], F32, name=f"sil_{ei}_{cc}",
                                    tag="x", bufs=2)
                    nc.scalar.activation(sil[:], gps[:], ACTF.Silu)
                    ut = ptmp.tile([128, T], F32, name=f"ut_{ei}_{cc}",
                                   tag="xTown", bufs=1)
                    nc.vector.tensor_tensor(ut[:], ups[:], cb[:], ALU.mult)
                    nc.gpsimd.tensor_tensor(
                        act_e[:, cc, :, :].rearrange("p a b -> p (a b)"),
                        sil[:], ut[:], ALU.mult)
                acts.append(act_e)

            # shared expert (column-parallel)
            act_sh = pbig.tile([128, TB, 128], BF16, name="act_sh",
                               tag="act_sh")
            gps_s = psA.tile([128, T], F32, name="gps_s", tag="A")
            ups_s = psA.tile([128, T], F32, name="ups_s", tag="A")
            for hc in range(HC):
                for col, ps in ((0, gps_s), (1, ups_s)):
                    for half in range(2):
                        nc.tensor.matmul(
                            ps[:, half * 512:(half + 1) * 512],
                            sw13_sb[:, hc, col * 128:(col + 1) * 128],
                            x2T[:, hc, half * 4:(half + 1) * 4, :],
                            start=(hc == 0), stop=(hc == HC - 1))
            sil_s = ptmp.tile([128, T], F32, name="sil_s", tag="x", bufs=2)
            nc.scalar.activation(sil_s[:], gps_s[:], ACTF.Silu)
            nc.vector.tensor_tensor(
                act_sh[:].rearrange("p a b -> p (a b)"), sil_s[:], ups_s[:],
                ALU.mult)

            if debug:
                nc.gpsimd.dma_start(
                    dbg["acta"][:],
                    acts[0][:].rearrange("p a b c -> p (a b c)"))

            # w2 stage: token-major output; 5 uneven column groups
            for g in range(len(W2G)):
                gw, go = W2G[g], W2OFF[g]
                w2g = []
                for ei, w2_d_ in enumerate((w2a_d, w2b_d)):
                    wt = pstream.tile([128, 4, gw], BF16,
                                      name=f"w2_{ei}_{g}", tag=f"w2g{ei}",
                                      bufs=2)
                    (nc.sync if ei == 0 else nc.scalar).dma_start(
                        wt[:], w2_d_[:, :, go:go + gw])
                    w2g.append(wt)
                for tb2 in range(TB):
                    ops = psA.tile([128, 512], F32, name=f"o_{g}_{tb2}",
                                   tag="A")
                    k = 0
                    for ei in range(2):
                        for ic in range(4):
                            nc.tensor.matmul(ops[:, 0:gw],
                                             acts[ei][:, ic, tb2, :],
                                             w2g[ei][:, ic, :],
                                             start=(k == 0), stop=False)
                            k += 1
                    nc.tensor.matmul(ops[:, 0:gw], act_sh[:, tb2, :],
                                     sw2_sb[:, go:go + gw],
                                     start=False, stop=True)
                    oo = ptmp.tile([128, 512], RS2_DT, name=f"oo_{g}_{tb2}",
                                   tag="dout", bufs=3)
                    if tb2 % 2 == 0:
                        nc.vector.tensor_copy(oo[:, 0:gw], ops[:, 0:gw])
                    else:
                        nc.scalar.activation(oo[:, 0:gw], ops[:, 0:gw],
                                             ACTF.Copy)
                    nc.gpsimd.dma_start(
                        rs2_in[g][tb2 * 128:(tb2 + 1) * 128, :],
                        oo[:, 0:gw])
                nc.gpsimd.collective_compute(
                    "AllToAll", ALU.bypass, replica_groups=rg,
                    ins=[rs2_in[g].opt()], outs=[a2a2_out[g].opt()])

            # local 8-way adds per column group, write f32 output directly
            for g in range(len(W2G)):
                gw, go = W2G[g], W2OFF[g]
                acc = psA.tile([128, 512], F32, name=f"acc_{g}", tag="A")
                for hf in range(2):
                    pg = ptmp.tile([128, 4, 512], RS2_DT,
                                   name=f"opart_{g}_{hf}", tag="pgh",
                                   bufs=2)
                    nc.sync.dma_start(
                        pg[:, :, 0:gw],
                        a2a2_out[g][hf * 4:(hf + 1) * 4].rearrange(
                            "a p b -> p a b"))
                    for i in range(4):
                        nc.tensor.matmul(acc[:, 0:gw], ident_b[:],
                                         pg[:, i, 0:gw],
                                         start=(hf == 0 and i == 0),
                                         stop=(hf == 1 and i == 3))
                og = ptmp.tile([128, 512], F32, name=f"og_{g}", tag="rope1",
                               bufs=2)
                nc.vector.tensor_copy(og[:, 0:gw], acc[:, 0:gw])
                nc.gpsimd.dma_start(out0_d[:, go:go + gw], og[:, 0:gw])

    nc.compile()
    return nc


def prep_in_maps(inputs):
    """Shard/marshal full inputs into 8 per-core input maps."""
    f32 = np.float32
    hid = np.asarray(inputs["hidden_states"], f32)
    res = np.asarray(inputs["residual"], f32)
    rms1 = np.asarray(inputs["rms1_w"], f32)
    rms2 = np.asarray(inputs["rms2_w"], f32)
    w_qkv = np.asarray(inputs["w_qkv"], f32) * rms1[:, None]
    w_dense = np.asarray(inputs["w_dense"], f32)
    gate_w = np.asarray(inputs["gate_w"], f32) * rms2[:, None]
    w13 = np.asarray(inputs["w13"], f32) * rms2[None, :, None]
    w2 = np.asarray(inputs["w2"], f32)
    sw13 = np.asarray(inputs["sw13"], f32) * rms2[:, None]
    sw2 = np.asarray(inputs["sw2"], f32)
    pos = np.asarray(inputs["position_ids"]).astype(f32)

    inv_freq = (1.0 / (ROPE_THETA **
                       (np.arange(0, D, 2, dtype=f32) / D))).astype(f32)
    ang = pos[:, None] * inv_freq[None, :]          # [T, 64]
    cosa = np.cos(ang).astype(f32)
    sina = np.sin(ang).astype(f32)
    s = np.float32(D ** -0.5)

    gate_dev = np.ascontiguousarray(
        gate_w.reshape(HC, 128, E).transpose(1, 0, 2))

    def bf(x):
        return np.ascontiguousarray(x.astype(BF16_NP))

    wkv_dev = bf(w_qkv[:, Q_SIZE:].reshape(HC, 128, 1024))
    wq_dev = bf(w_qkv[:, :Q_SIZE].reshape(HC, 128, Q_SIZE))
    wd_dev = bf(w_dense.reshape(HC, 128, H))

    in_maps = []
    for c in range(N_CORES):
        tok = slice(128 * c, 128 * c + 128)
        co = np.tile(cosa[tok], (1, 8)).reshape(128, 8, 64)
        si = np.tile(sina[tok], (1, 8)).reshape(128, 8, 64)
        causal = (np.arange(T)[None, :]
                  <= (128 * c + np.arange(128))[:, None])
        maskb = np.where(causal, -4.0, -60.0).astype(f32)

        sw13_c = np.concatenate(
            [sw13[:, 128 * c:128 * c + 128],
             sw13[:, SI + 128 * c:SI + 128 * c + 128]], 1)  # [2048, 256]
        sw13_dev = bf(sw13_c.reshape(HC, 128, 256).transpose(1, 0, 2))
        sw2_dev = bf(sw2[128 * c:128 * c + 128, :])       # [128, 2048]

        def w13_dev(e):
            m = w13[e]                                    # [2048, 1024]
            return bf(m.reshape(HC, 128, 8, 128).transpose(2, 1, 0, 3)
                      .reshape(8, 128, H))

        def w2_dev(e):
            m = w2[e]                                     # [512, 2048]
            return bf(m.reshape(4, 128, H).transpose(1, 0, 2))

        sel = np.zeros((2, E, 128), f32)
        sel[0, 2 * c, :] = 1.0
        sel[1, 2 * c + 1, :] = 1.0

        in_maps.append({
            "resid_own": np.ascontiguousarray(hid[tok] + res[tok]),
            "wkv": wkv_dev, "wq": wq_dev, "wdense": wd_dev,
            "sw13": sw13_dev, "sw2": sw2_dev,
            "w13a": w13_dev(2 * c), "w13b": w13_dev(2 * c + 1),
            "w2a": w2_dev(2 * c), "w2b": w2_dev(2 * c + 1),
            "gate": gate_dev,
            "qcos": np.ascontiguousarray(co * s),
            "qsin": np.ascontiguousarray(si * s),
            "kcos": np.ascontiguousarray(co[:, 0:4, :]),
            "ksin": np.ascontiguousarray(si[:, 0:4, :]),
            "maskb": bf(maskb),
            "sela": np.ascontiguousarray(sel[0]),
            "selb": np.ascontiguousarray(sel[1]),
        })
    return in_maps


_NC_CACHE = {}


def _get_nc(debug=False):
    key = debug
    if key not in _NC_CACHE:
        _NC_CACHE[key] = build_nc(debug=debug)
    return _NC_CACHE[key]


def run(inputs, debug=False, trace=False):
    nc = _get_nc(debug=debug)
    in_maps = prep_in_maps(inputs)
    kw = {}
    if trace:
        kw["trace"] = True
    res = run_bass_kernel_spmd(nc, in_maps, core_ids=list(range(N_CORES)),
                               **kw)
    out0 = np.concatenate([res.results[c]["out0"] for c in range(N_CORES)], 0)
    out1 = np.concatenate([res.results[c]["out1"] for c in range(N_CORES)], 0)
    return (out0, out1), res


def kernel(**inputs):
    (out0, out1), _ = run(inputs)
    return out0, out1


# revision 38
# speedup vs baseline: 1.0175x; 1.0017x over previous
"""Trainium2 Bass kernel for nn_BailingMoeBlock (8 NeuronCores).

Sharding (v5):
  - rmsnorm1: transposes run on RAW resid (bf16); 1/rms folded into the
    rope cos/sin tables and the v copy, so the PE never waits on the
    variance reduction.
  - Attention: token-parallel (own 128 q x all 1024 kv); single kv
    AllGather.  Softmax: additive causal mask (-4 exp-bias folded in, -60
    masked), exp with accum_out row-sums, DVE renormalize.  Heads run in
    kv-sharing pairs; pair context = one N=256 matmul per kv block;
    depth-2 head pipeline.  All staging DMAs stay on the sync/scalar
    hardware-DGE queues (gpsimd SWDGE staging faults on this runtime).
  - resid2 / rmsnorm2 / router on the owned block; x2^T + router weights
    leave in two chunked AllGathers (hc 0..9, then hc 10..15 + comb).
  - MoE: expert-parallel (2 experts/core), hc-outer/half-inner loops so
    each LDWEIGHTS serves two N=512 matmuls; the combine-weight broadcast
    matmul sits inside cc0 (not at the phase head) so it cannot block
    AG-covered work.  Shared expert column-parallel.  w2 computes 4
    column sub-groups of 512 and ships them in 3 decreasing AllToAlls
    (1024, 512, 512 cols): the collective backlog drains inside the
    compute window and only the small last wire is exposed.  Local 8-way
    adds via identity matmuls.

Heavy matmuls in bf16 (fp32 accumulate in PSUM); router in fp32.
"""
import sys
import numpy as np

for _p in ("/opt/trn_rl_repo", "/opt/pypackages"):
    if _p not in sys.path:
        sys.path.append(_p)

import ml_dtypes  # noqa: E402
from concourse import bacc  # noqa: E402
import concourse.bass as bass  # noqa: E402
import concourse.tile as tile  # noqa: E402
import concourse.mybir as mybir  # noqa: E402
from concourse.bass_utils import run_bass_kernel_spmd  # noqa: E402
from concourse.masks import make_identity  # noqa: E402

F32 = mybir.dt.float32
BF16 = mybir.dt.bfloat16
BF16_NP = ml_dtypes.bfloat16

N_CORES = 8
T = 1024
H = 2048
NH = 16
NKV = 4
D = 128
E = 16
I_EXP = 512
SI = 1024
TOPK = 4
RMS_EPS = 1e-6
ROPE_THETA = 10000.0
Q_SIZE = NH * D          # 2048
KV_SIZE = NKV * D        # 512
TB = T // 128            # 8 token blocks
HC = H // 128            # 16 h chunks
NEG_BIG = -30000.0
HCA = 10                 # x2T chunks in first AG1 chunk
HCB = HC - HCA           # remaining chunks (ride with router weights)
W2G = (512, 512, 512, 384, 128)   # w2 column-group widths
W2OFF = (0, 512, 1024, 1536, 1920)

RS2_DT = BF16

X = mybir.AxisListType.X
ALU = mybir.AluOpType
ACTF = mybir.ActivationFunctionType


def build_nc(debug=False):
    nc = bacc.Bacc("TRN2", target_bir_lowering=False, debug=False,
                   num_devices=N_CORES)

    def din(name, shape, dt):
        return nc.dram_tensor(name, list(shape), dt, kind="ExternalInput").ap()

    def dout(name, shape, dt):
        return nc.dram_tensor(name, list(shape), dt, kind="ExternalOutput").ap()

    wkv_d = din("wkv", (HC, 128, 1024), BF16)    # k|v columns of w_qkv
    wq_d = din("wq", (HC, 128, 2048), BF16)      # q columns of w_qkv
    wd_d = din("wdense", (HC, 128, H), BF16)     # full w_dense, chunk-major
    sw13_d = din("sw13", (128, HC, 256), BF16)
    sw2_d = din("sw2", (128, H), BF16)
    w13a_d = din("w13a", (8, 128, H), BF16)   # [cc][p][hc*128+ci]
    w13b_d = din("w13b", (8, 128, H), BF16)
    w2a_d = din("w2a", (128, 4, H), BF16)
    w2b_d = din("w2b", (128, 4, H), BF16)
    gate_d = din("gate", (128, HC, E), F32)
    resid_own_d = din("resid_own", (128, H), F32)  # host hid+res own block
    qcos_d = din("qcos", (128, 8, 64), F32)   # own-token cos*scale, x8 heads
    qsin_d = din("qsin", (128, 8, 64), F32)
    kcos_d = din("kcos", (128, 4, 64), F32)
    ksin_d = din("ksin", (128, 4, 64), F32)
    maskb_d = din("maskb", (128, T), BF16)    # additive mask: -4 / -30004
    sela_d = din("sela", (E, 128), F32)
    selb_d = din("selb", (E, 128), F32)

    out0_d = dout("out0", (128, H), F32)
    out1_d = dout("out1", (128, H), F32)
    dbg = {}
    if debug:
        dbg["q"] = dout("dbg_q", (128, 128), F32)        # qT head0 (own toks)
        dbg["att"] = dout("dbg_att", (128, H), F32)      # attn_out own block
        dbg["x2"] = dout("dbg_x2", (128, H), F32)        # x2 own block fp32
        dbg["comb"] = dout("dbg_comb", (128, E), F32)    # comb own block
        dbg["acta"] = dout("dbg_acta", (128, 4 * T), F32)  # act expert a

    with tile.TileContext(nc) as tc:
        with (
            tc.tile_pool(name="const", bufs=1) as pc,
            tc.tile_pool(name="weights", bufs=1) as pw,
            tc.tile_pool(name="big", bufs=1) as pbig,
            tc.tile_pool(name="stream", bufs=3) as pstream,
            tc.tile_pool(name="tmp", bufs=2) as ptmp,
            tc.tile_pool(name="psA", bufs=3, space="PSUM") as psA,
            tc.tile_pool(name="psB", bufs=2, space="PSUM") as psB,
            tc.tile_pool(name="dram", bufs=1, space="DRAM") as pd,
        ):
            # ---------------- constants (scalar/ACT DMA queue) -------------
            ident_b = pc.tile([128, 128], BF16, name="ident_b")
            make_identity(nc, ident_b[:])
            ident_f = pc.tile([128, 128], F32, name="ident_f")
            make_identity(nc, ident_f[:])
            qcos = pc.tile([128, 8, 64], F32, name="qcos")
            nc.scalar.dma_start(qcos[:], qcos_d[:])
            qsin = pc.tile([128, 8, 64], F32, name="qsin")
            nc.scalar.dma_start(qsin[:], qsin_d[:])
            kcos = pc.tile([128, 4, 64], F32, name="kcos")
            nc.scalar.dma_start(kcos[:], kcos_d[:])
            ksin = pc.tile([128, 4, 64], F32, name="ksin")
            nc.scalar.dma_start(ksin[:], ksin_d[:])
            maskb = pc.tile([128, T], BF16, name="maskb")
            nc.scalar.dma_start(maskb[:], maskb_d[:])
            gate_sb = pc.tile([128, HC, E], F32, name="gate_sb")
            nc.scalar.dma_start(gate_sb[:], gate_d[:])
            sela_sb = pc.tile([E, 128], F32, name="sela_sb")
            nc.scalar.dma_start(sela_sb[:], sela_d[:])
            selb_sb = pc.tile([E, 128], F32, name="selb_sb")
            nc.scalar.dma_start(selb_sb[:], selb_d[:])

            # ---------------- DRAM internal buffers ----------------
            agkv_in = pd.tile([128, 1024], BF16, name="agkv_in")
            agkv_out = pd.tile([N_CORES, 128, 1024], BF16, name="agkv_out",
                               addr_space="Shared")
            ag1a_in = pd.tile([128, HCA * 128], BF16, name="ag1a_in")
            ag1a_out = pd.tile([TB, 128, HCA * 128], BF16,
                               name="ag1a_out", addr_space="Shared")
            ag1b_in = pd.tile([128, HCB * 128 + E], BF16, name="ag1b_in")
            ag1b_out = pd.tile([TB, 128, HCB * 128 + E], BF16,
                               name="ag1b_out", addr_space="Shared")
            A2AG = ((0, 1024), (1024, 512), (1536, 512))
            rs2_in = [pd.tile([T, gw], RS2_DT, name=f"rs2_in{g}")
                      for g, (go, gw) in enumerate(A2AG)]
            a2a2_out = [pd.tile([TB, 128, gw], RS2_DT,
                                name=f"a2a2_out{g}")
                        for g, (go, gw) in enumerate(A2AG)]
            rg = [list(range(N_CORES))]

            # ===== P0: own-block resid; transposes on RAW resid =====
            resid_own = pbig.tile([128, H], F32, name="resid_own", tag="ro")
            nc.sync.dma_start(resid_own[:, 0:1024], resid_own_d[:, 0:1024])
            nc.sync.dma_start(resid_own[:, 1024:2048],
                              resid_own_d[:, 1024:2048])
            rb = ptmp.tile([128, H], BF16, name="rb", tag="x", bufs=2)
            nc.scalar.activation(rb[:, 0:1024], resid_own[:, 0:1024],
                                 ACTF.Copy)
            nc.scalar.activation(rb[:, 1024:2048], resid_own[:, 1024:2048],
                                 ACTF.Copy)
            # variance (off the transpose critical path)
            sqj = ptmp.tile([128, H], BF16, name="sqj", tag="x", bufs=2)
            ssum0 = ptmp.tile([128, 1], F32, name="ssum0", tag="nrm", bufs=4)
            nc.scalar.activation(sqj[:], resid_own[:], ACTF.Square,
                                 accum_out=ssum0[:])
            var0 = ptmp.tile([128, 1], F32, name="var0", tag="nrm", bufs=4)
            nc.vector.tensor_scalar(var0[:], ssum0[:], 1.0 / H, RMS_EPS,
                                    ALU.mult, ALU.add)
            sd0 = ptmp.tile([128, 1], F32, name="sd0", tag="nrm", bufs=4)
            nc.scalar.activation(sd0[:], var0[:], ACTF.Sqrt)
            rstd0 = ptmp.tile([128, 1], F32, name="rstd0", tag="nrm", bufs=4)
            nc.vector.reciprocal(rstd0[:], sd0[:])
            # rstd-folded rope tables (one tile; overlays P6's cb slot)
            cosR_all = ptmp.tile([128, 24, 64], F32, name="cosR_all",
                                 tag="cb", bufs=1)
            qcosR = cosR_all[:, 0:8, :]
            qsinR = cosR_all[:, 8:16, :]
            kcosR = cosR_all[:, 16:20, :]
            ksinR = cosR_all[:, 20:24, :]
            nc.vector.tensor_scalar_mul(qcosR, qcos[:], rstd0[:])
            nc.vector.tensor_scalar_mul(qsinR, qsin[:], rstd0[:])
            nc.vector.tensor_scalar_mul(kcosR, kcos[:], rstd0[:])
            nc.vector.tensor_scalar_mul(ksinR, ksin[:], rstd0[:])

            xT_raw = ptmp.tile([128, HC, 128], BF16, name="xT_raw",
                               tag="xTown", bufs=1)
            for hg in range(4):
                tp = psB.tile([128, 4, 128], BF16, name=f"tpx_{hg}", tag="B")
                for j in range(4):
                    hcc = hg * 4 + j
                    nc.tensor.transpose(
                        tp[:, j, :], rb[:, hcc * 128:(hcc + 1) * 128],
                        ident_b[:])
                if hg % 2 == 0:
                    nc.vector.tensor_copy(xT_raw[:, hg * 4:(hg + 1) * 4, :],
                                          tp[:])
                else:
                    nc.scalar.activation(xT_raw[:, hg * 4:(hg + 1) * 4, :],
                                         tp[:], ACTF.Copy)

            # ===== P1a: kv projection (raw) + rstd-folded rope + AGk/AGv ===
            pskv = psA.tile([128, 1024], F32, name="pskv", tag="A")
            for hc in range(HC):
                wkvc = pstream.tile([128, 1024], BF16, name=f"wkv_{hc}",
                                    tag="wstr", bufs=4)
                (nc.sync if hc % 2 == 0 else nc.scalar).dma_start(
                    wkvc[:], wkv_d[hc])
                for c2 in range(2):
                    nc.tensor.matmul(
                        pskv[:, c2 * 512:(c2 + 1) * 512], xT_raw[:, hc, :],
                        wkvc[:, c2 * 512:(c2 + 1) * 512],
                        start=(hc == 0), stop=(hc == HC - 1))

            def rope_tok(pview, cost, sint, dst, nh):
                """pview [128, nh, 128] psum; dst [128, nh, 128] bf16 sbuf."""
                x1 = pview[:, :, 0:64]
                x2 = pview[:, :, 64:128]
                ta = ptmp.tile([128, nh, 64], F32, name="ta", tag="rope1",
                               bufs=2)
                tb = ptmp.tile([128, nh, 64], F32, name="tb", tag="rope2",
                               bufs=2)
                tc2 = ptmp.tile([128, nh, 64], F32, name="tc", tag="rope1",
                                bufs=2)
                td = ptmp.tile([128, nh, 64], F32, name="td", tag="rope2",
                               bufs=2)
                nc.vector.tensor_tensor(ta[:], x1, cost, ALU.mult)
                nc.vector.tensor_tensor(tb[:], x2, sint, ALU.mult)
                nc.vector.tensor_tensor(tc2[:], x2, cost, ALU.mult)
                nc.vector.tensor_tensor(td[:], x1, sint, ALU.mult)
                nc.vector.tensor_tensor(dst[:, :, 0:64], ta[:], tb[:],
                                        ALU.subtract)
                nc.vector.tensor_tensor(dst[:, :, 64:128], tc2[:], td[:],
                                        ALU.add)

            k_own = ptmp.tile([128, NKV, 128], BF16, name="k_own",
                              tag="kown", bufs=1)
            rope_tok(pskv[:, 0:512].rearrange("p (h d) -> p h d", h=NKV),
                     kcosR, ksinR, k_own, NKV)
            v_own = ptmp.tile([128, 512], BF16, name="v_own", tag="vb",
                              bufs=1)
            nc.scalar.activation(v_own[:], pskv[:, 512:1024], ACTF.Copy,
                                 scale=rstd0[:])
            kT_own = ptmp.tile([128, NKV, 128], BF16, name="kT_own",
                               tag="cp", bufs=1)
            tpk = psB.tile([128, 4, 128], BF16, name="tpk", tag="B")
            for j in range(NKV):
                nc.tensor.transpose(tpk[:, j, :], k_own[:, j, :], ident_b[:])
            nc.vector.tensor_copy(kT_own[:], tpk[:])
            nc.gpsimd.dma_start(agkv_in[:, 0:512],
                                kT_own[:].rearrange("p a b -> p (a b)"))
            nc.gpsimd.dma_start(agkv_in[:, 512:1024], v_own[:])
            nc.gpsimd.collective_compute(
                "AllGather", ALU.bypass, replica_groups=rg,
                ins=[agkv_in.opt()], outs=[agkv_out.opt()])

            # ===== P1b: q projection (raw) + rstd-folded rope + qT =====
            q_own = ptmp.tile([128, NH, 128], BF16, name="q_own", tag="x",
                              bufs=2)
            for pg in range(2):
                psq = psA.tile([128, 1024], F32, name=f"psq_{pg}", tag="A")
                for hc in range(HC):
                    wqc = pstream.tile([128, 1024], BF16,
                                       name=f"wq_{pg}_{hc}", tag="wstr",
                                       bufs=4)
                    (nc.sync if hc % 2 == 0 else nc.scalar).dma_start(
                        wqc[:], wq_d[hc, :, pg * 1024:(pg + 1) * 1024])
                    for c2 in range(2):
                        nc.tensor.matmul(
                            psq[:, c2 * 512:(c2 + 1) * 512],
                            xT_raw[:, hc, :],
                            wqc[:, c2 * 512:(c2 + 1) * 512],
                            start=(hc == 0), stop=(hc == HC - 1))
                rope_tok(psq[:].rearrange("p (h d) -> p h d", h=8),
                         qcosR, qsinR,
                         q_own[:, pg * 8:(pg + 1) * 8, :], 8)
            qT = ptmp.tile([128, NH, 128], BF16, name="qT", tag="qT", bufs=1)
            for hg in range(4):
                tpq = psB.tile([128, 4, 128], BF16, name=f"tpq_{hg}",
                               tag="B")
                for j in range(4):
                    nc.tensor.transpose(tpq[:, j, :], q_own[:, hg * 4 + j, :],
                                        ident_b[:])
                nc.vector.tensor_copy(qT[:, hg * 4:(hg + 1) * 4, :], tpq[:])

            # ===== P1c: gather k/v of all tokens (k first) =====
            kT_full = pbig.tile([128, TB, NKV, 128], BF16, name="kT_full",
                                tag="kT")
            v_sb = pbig.tile([128, TB, NKV, 128], BF16, name="v_sb",
                             tag="vsb")
            for hf in range(2):
                cs = slice(4 * hf, 4 * hf + 4)
                (nc.sync if hf == 0 else nc.scalar).dma_start(
                    kT_full[:, cs, :, :],
                    agkv_out[cs, :, 0:512].rearrange(
                        "c p (a b) -> p c a b", a=NKV))
            for hf in range(2):
                cs = slice(4 * hf, 4 * hf + 4)
                (nc.sync if hf == 0 else nc.scalar).dma_start(
                    v_sb[:, cs, :, :],
                    agkv_out[cs, :, 512:1024].rearrange(
                        "c p (a b) -> p c a b", a=NKV))

            # prefetch w_dense + shared-expert weights (queues idle now)
            wd_sb = pbig.tile([128, 8, H], BF16, name="wd_sb", tag="xT")
            for ch in range(8):
                (nc.sync if ch % 2 == 0 else nc.scalar).dma_start(
                    wd_sb[:, ch, :], wd_d[ch])
            sw13_sb = pw.tile([128, HC, 256], BF16, name="sw13_sb", tag="wA")
            nc.sync.dma_start(sw13_sb[:], sw13_d[:])
            sw2_sb = pw.tile([128, H], BF16, name="sw2_sb")
            nc.scalar.dma_start(sw2_sb[:], sw2_d[:])

            # ===== P2: attention, 16 heads in kv-sharing pairs =====
            ctxga = pbig.tile([128, NH, 128], BF16, name="ctxga",
                              tag="ctxga")

            def head_scores(h):
                sc = psA.tile([128, T], F32, name=f"sc_{h}", tag="A")
                for c2 in range(2):
                    nc.tensor.matmul(
                        sc[:, c2 * 512:(c2 + 1) * 512], qT[:, h, :],
                        kT_full[:, c2 * 4:(c2 + 1) * 4, h // 4, :],
                        start=True, stop=True)
                sb = ptmp.tile([128, T], BF16, name=f"sb_{h}", tag="wds",
                               bufs=2)
                nc.vector.tensor_tensor(sb[:], sc[:], maskb[:], ALU.add)
                pb = ptmp.tile([128, T], BF16, name=f"pb_{h}", tag="pb",
                               bufs=4)
                rs_ = ptmp.tile([128, 1], F32, name="rs", tag="negm", bufs=8)
                nc.scalar.activation(pb[:], sb[:], ACTF.Exp,
                                     accum_out=rs_[:])
                rinv = ptmp.tile([128, 1], F32, name="rinv", tag="negm",
                                 bufs=8)
                nc.vector.reciprocal(rinv[:], rs_[:])
                nc.vector.tensor_scalar_mul(pb[:], pb[:], rinv[:])
                return pb

            def pair_ctx(p, pb0, pb1):
                """Context for heads 2p, 2p+1 (share kv head p//2)."""
                pT = ptmp.tile([128, TB, 2, 128], BF16, name=f"pT_{p}",
                               tag=f"probsT{p % 2}", bufs=1)
                for hi, pb in ((0, pb0), (1, pb1)):
                    for g4 in range(2):
                        tpp = psB.tile([128, 4, 128], BF16,
                                       name=f"tpp_{p}_{hi}_{g4}", tag="B")
                        for j in range(4):
                            b = g4 * 4 + j
                            nc.tensor.transpose(
                                tpp[:, j, :],
                                pb[:, b * 128:(b + 1) * 128],
                                ident_b[:])
                        eng = nc.vector if (hi + g4) % 2 == 0 else None
                        if eng is not None:
                            eng.tensor_copy(
                                pT[:, g4 * 4:(g4 + 1) * 4, hi, :], tpp[:])
                        else:
                            nc.scalar.activation(
                                pT[:, g4 * 4:(g4 + 1) * 4, hi, :], tpp[:],
                                ACTF.Copy)
                cps = psB.tile([128, 2, 128], F32, name=f"cx_{p}", tag="B")
                for b in range(TB):
                    nc.tensor.matmul(cps[:], v_sb[:, b, p // 2, :],
                                     pT[:, b, :, :],
                                     start=(b == 0), stop=(b == TB - 1))
                if p % 2 == 0:
                    nc.vector.tensor_copy(ctxga[:, 2 * p:2 * p + 2, :],
                                          cps[:])
                else:
                    nc.scalar.activation(ctxga[:, 2 * p:2 * p + 2, :],
                                         cps[:], ACTF.Copy)

            pbs = []
            for h in range(NH):
                pbs.append(head_scores(h))
                if h % 2 == 1 and h >= 3:
                    p = (h - 3) // 2
                    pair_ctx(p, pbs[2 * p], pbs[2 * p + 1])
            pair_ctx(7, pbs[14], pbs[15])

            if debug:
                nc.gpsimd.dma_start(dbg["q"][:], qT[:, 0, :])
            # ============ P3: local dense projection ============
            dh = [psA.tile([128, 1024], F32, name=f"dh_{i}", tag="A")
                  for i in range(2)]
            for ch in range(HC):
                if ch < 8:
                    wh = [wd_sb[:, ch, 0:1024], wd_sb[:, ch, 1024:2048]]
                else:
                    wh = []
                    for half in range(2):
                        wt = pstream.tile([128, 1024], BF16,
                                          name=f"wdc_{ch}_{half}",
                                          tag="wstr", bufs=4)
                        (nc.sync if half == 0 else nc.scalar).dma_start(
                            wt[:], wd_d[ch, :, half * 1024:(half + 1) * 1024])
                        wh.append(wt[:])
                for half in range(2):
                    for c2 in range(2):
                        nc.tensor.matmul(
                            dh[half][:, c2 * 512:(c2 + 1) * 512],
                            ctxga[:, ch, :],
                            wh[half][:, c2 * 512:(c2 + 1) * 512],
                            start=(ch == 0), stop=(ch == HC - 1))

            # ============ P4: resid2, rmsnorm2, router, chunked AG ========
            resid2 = pstream.tile([128, H], F32, name="resid2", tag="hr",
                                  bufs=1)
            nc.vector.tensor_tensor(resid2[:, 0:1024], dh[0][:],
                                    resid_own[:, 0:1024], ALU.add)
            nc.vector.tensor_tensor(resid2[:, 1024:2048], dh[1][:],
                                    resid_own[:, 1024:2048], ALU.add)
            if debug:
                att_f = ptmp.tile([128, H], F32, name="att_f", tag="attdbg",
                                  bufs=1)
                nc.vector.tensor_tensor(att_f[:], resid2[:], resid_own[:],
                                        ALU.subtract)
                nc.gpsimd.dma_start(dbg["att"][:], att_f[:])

            ssum4 = ptmp.tile([128, 1], F32, name="ssum4", tag="nrm", bufs=4)
            sq4 = ptmp.tile([128, H], BF16, name="sq4", tag="x", bufs=2)
            nc.scalar.activation(sq4[:], resid2[:], ACTF.Square,
                                 accum_out=ssum4[:])
            var4 = ptmp.tile([128, 1], F32, name="var4", tag="nrm", bufs=4)
            nc.vector.tensor_scalar(var4[:], ssum4[:], 1.0 / H, RMS_EPS,
                                    ALU.mult, ALU.add)
            sd4 = ptmp.tile([128, 1], F32, name="sd4", tag="nrm", bufs=4)
            nc.scalar.activation(sd4[:], var4[:], ACTF.Sqrt)
            rstd4 = ptmp.tile([128, 1], F32, name="rstd4", tag="nrm", bufs=4)
            nc.vector.reciprocal(rstd4[:], sd4[:])

            x2_f = pbig.tile([128, H], F32, name="x2_f", tag="kT")
            nc.scalar.activation(x2_f[:, 0:HCA * 128],
                                 resid2[:, 0:HCA * 128], ACTF.Copy,
                                 scale=rstd4[:])
            nc.scalar.activation(x2_f[:, HCA * 128:H],
                                 resid2[:, HCA * 128:H], ACTF.Copy,
                                 scale=rstd4[:])
            if debug:
                nc.gpsimd.dma_start(dbg["x2"][:], x2_f[:])
            x2Tf = pbig.tile([128, HC, 128], F32, name="x2Tf", tag="x2Tf")
            x2T_own = ptmp.tile([128, HC, 128], BF16, name="x2T_own",
                                tag="xTown", bufs=1)
            for hg in range(8):
                tp = psB.tile([128, 2, 128], F32, name=f"tpn_{hg}", tag="B")
                for j in range(2):
                    hcc = hg * 2 + j
                    nc.tensor.transpose(
                        tp[:, j, :], x2_f[:, hcc * 128:(hcc + 1) * 128],
                        ident_f[:])
                nc.vector.tensor_copy(x2Tf[:, hg * 2:(hg + 1) * 2, :],
                                      tp[:])
                nc.scalar.activation(x2T_own[:, hg * 2:(hg + 1) * 2, :],
                                     tp[:], ACTF.Copy)
            nc.gpsimd.dma_start(
                ag1_in[:, 0:HC * 128],
                x2T_own[:].rearrange("p a b -> p (a b)"))
            nc.gpsimd.dma_start(out1_d[:], resid2[:])

            # router on own block
            lg = psB.tile([128, E], F32, name="lg", tag="B")
            for hc in range(HC):
                nc.tensor.matmul(lg[:], x2Tf[:, hc, :], gate_sb[:, hc, :],
                                 start=(hc == 0), stop=(hc == HC - 1))
            negm1 = ptmp.tile([128, 1], F32, name="negm1", tag="negm",
                              bufs=8)
            nc.vector.reduce_max(negm1[:], lg[:], axis=X, negate=True)
            ee = ptmp.tile([128, E], F32, name="ee", tag="t_ee", bufs=2)
            nc.scalar.activation(ee[:], lg[:], ACTF.Exp, bias=negm1[:])
            work = ptmp.tile([128, E], F32, name="work", tag="t_wk", bufs=2)
            nc.vector.tensor_copy(work[:], ee[:])
            mth = ptmp.tile([128, 1], F32, name="mth", tag="negm", bufs=8)
            nc.vector.reduce_max(mth[:], work[:], axis=X)
            msk = ptmp.tile([128, E], F32, name="msk", tag="t_mk", bufs=2)
            for _ in range(TOPK - 1):
                nc.vector.tensor_scalar(msk[:], work[:], mth[:], 1e30,
                                        ALU.is_ge, ALU.mult)
                nc.vector.tensor_tensor(work[:], work[:], msk[:],
                                        ALU.subtract)
                nc.vector.reduce_max(mth[:], work[:], axis=X)
            ge = ptmp.tile([128, E], F32, name="ge", tag="t_ge", bufs=2)
            nc.vector.tensor_scalar(ge[:], ee[:], mth[:], None, ALU.is_ge)
            cu = ptmp.tile([128, E], F32, name="cu", tag="t_cu", bufs=2)
            nc.vector.tensor_tensor(cu[:], ee[:], ge[:], ALU.mult)
            s4 = ptmp.tile([128, 1], F32, name="s4", tag="negm", bufs=8)
            nc.vector.reduce_sum(s4[:], cu[:], axis=X)
            ri4 = ptmp.tile([128, 1], F32, name="ri4", tag="negm", bufs=8)
            nc.vector.reciprocal(ri4[:], s4[:])
            combo = ptmp.tile([128, E], BF16, name="combo", tag="t_cb",
                              bufs=2)
            nc.vector.tensor_scalar_mul(combo[:], cu[:], ri4[:])
            nc.gpsimd.dma_start(ag1_in[:, HC * 128:HC * 128 + E],
                                combo[:])
            nc.gpsimd.collective_compute(
                "AllGather", ALU.bypass, replica_groups=rg,
                ins=[ag1_in.opt()], outs=[ag1_out.opt()])

            # ============ P5: unpack x2T + combT from chunked AGs ========
            x2T = pbig.tile([128, HC, TB, 128], BF16, name="x2T", tag="xT")
            for tb2 in range(TB):
                nc.sync.dma_start(
                    x2T[:, 0:HCA, tb2, :],
                    ag1a_out[tb2].rearrange("p (a b) -> p a b", a=HCA))
            cp_sb = ptmp.tile([128, TB, E], BF16, name="cp_sb", tag="cp",
                              bufs=1)
            nc.scalar.dma_start(
                cp_sb[:],
                ag1b_out[:, :, HCB * 128:HCB * 128 + E].rearrange(
                    "a p e -> p a e"))
            for tb2 in range(TB):
                nc.scalar.dma_start(
                    x2T[:, HCA:HC, tb2, :],
                    ag1b_out[tb2, :, 0:HCB * 128].rearrange(
                        "p (a b) -> p a b", a=HCB))
            combT = ptmp.tile([E, TB, 128], F32, name="combT", tag="qT",
                              bufs=1)
            for tb2 in range(TB):
                tpc = psB.tile([E, 128], BF16, name=f"tpc_{tb2}", tag="B")
                nc.tensor.transpose(tpc[:], cp_sb[:, tb2, :], ident_b[:])
                nc.vector.tensor_copy(combT[:, tb2, :], tpc[:])

            # ============ P6: experts + shared ============
            acts = []
            for ei, (w13_d_, sel_sb) in enumerate(
                    ((w13a_d, sela_sb), (w13b_d, selb_sb))):
                act_e = pbig.tile([128, 4, TB, 128], BF16, name=f"act_{ei}",
                                  tag=("x2Tf" if ei == 0 else "ro"))
                for cc in range(4):
                    wt_g = pstream.tile([128, HC, 128], BF16,
                                        name=f"wg_{ei}_{cc}", tag="w13",
                                        bufs=4)
                    nc.sync.dma_start(wt_g[:], w13_d_[cc].rearrange(
                        "p (a b) -> p a b", a=HC))
                    wt_u = pstream.tile([128, HC, 128], BF16,
                                        name=f"wu_{ei}_{cc}", tag="w13",
                                        bufs=4)
                    nc.scalar.dma_start(wt_u[:], w13_d_[cc + 4].rearrange(
                        "p (a b) -> p a b", a=HC))
                    gps = psA.tile([128, T], F32, name=f"g_{ei}_{cc}",
                                   tag="A")
                    ups = psA.tile([128, T], F32, name=f"u_{ei}_{cc}",
                                   tag="A")
                    for hc in range(HC):
                        for half in range(2):
                            nc.tensor.matmul(
                                gps[:, half * 512:(half + 1) * 512],
                                wt_g[:, hc, :],
                                x2T[:, hc, half * 4:(half + 1) * 4, :],
                                start=(hc == 0), stop=(hc == HC - 1))
                    if cc == 0:
                        bps = psA.tile([128, T], F32, name=f"bps_{ei}",
                                       tag="A")
                        for half in range(2):
                            nc.tensor.matmul(
                                bps[:, half * 512:(half + 1) * 512],
                                sel_sb[:],
                                combT[:, half * 4:(half + 1) * 4, :],
                                start=True, stop=True)
                        cb = ptmp.tile([128, T], F32, name=f"cb_{ei}",
                                       tag="cb", bufs=1)
                        nc.vector.tensor_copy(cb[:], bps[:])
                    for hc in range(HC):
                        for half in range(2):
                            nc.tensor.matmul(
                                ups[:, half * 512:(half + 1) * 512],
                                wt_u[:, hc, :],
                                x2T[:, hc, half * 4:(half + 1) * 4, :],
                                start=(hc == 0), stop=(hc == HC - 1))
                    sil = ptmp.tile([128, T], F32, name=f"sil_{ei}_{cc}",
                                    tag="x", bufs=2)
                    nc.scalar.activation(sil[:], gps[:], ACTF.Silu)
                    ut = ptmp.tile([128, T], F32, name=f"ut_{ei}_{cc}",
                                   tag="xTown", bufs=1)
                    nc.vector.tensor_tensor(ut[:], ups[:], cb[:], ALU.mult)
                    nc.gpsimd.tensor_tensor(
                        act_e[:, cc, :, :].rearrange("p a b -> p (a b)"),
                        sil[:], ut[:], ALU.mult)
                acts.append(act_e)

            # shared expert (column-parallel)
            act_sh = pbig.tile([128, TB, 128], BF16, name="act_sh",
                               tag="act_sh")
            gps_s = psA.tile([128, T], F32, name="gps_s", tag="A")
            ups_s = psA.tile([128, T], F32, name="ups_s", tag="A")
            for hc in range(HC):
                for col, ps in ((0, gps_s), (1, ups_s)):
                    for half in range(2):
                        nc.tensor.matmul(
                            ps[:, half * 512:(half + 1) * 512],
                            sw13_sb[:, hc, col * 128:(col + 1) * 128],
                            x2T[:, hc, half * 4:(half + 1) * 4, :],
                            start=(hc == 0), stop=(hc == HC - 1))
            sil_s = ptmp.tile([128, T], F32, name="sil_s", tag="x", bufs=2)
            nc.scalar.activation(sil_s[:], gps_s[:], ACTF.Silu)
            nc.vector.tensor_tensor(
                act_sh[:].rearrange("p a b -> p (a b)"), sil_s[:], ups_s[:],
                ALU.mult)

            if debug:
                nc.gpsimd.dma_start(
                    dbg["acta"][:],
                    acts[0][:].rearrange("p a b c -> p (a b c)"))

            # w2 stage: token-major output; 5 uneven column groups
            for g in range(len(W2G)):
                gw, go = W2G[g], W2OFF[g]
                w2g = []
                for ei, w2_d_ in enumerate((w2a_d, w2b_d)):
                    wt = pstream.tile([128, 4, gw], BF16,
                                      name=f"w2_{ei}_{g}", tag=f"w2g{ei}",
                                      bufs=2)
                    (nc.sync if ei == 0 else nc.scalar).dma_start(
                        wt[:], w2_d_[:, :, go:go + gw])
                    w2g.append(wt)
                for tb2 in range(TB):
                    ops = psA.tile([128, 512], F32, name=f"o_{g}_{tb2}",
                                   tag="A")
                    k = 0
                    for ei in range(2):
                        for ic in range(4):
                            nc.tensor.matmul(ops[:, 0:gw],
                                             acts[ei][:, ic, tb2, :],
                                             w2g[ei][:, ic, :],
                                             start=(k == 0), stop=False)
                            k += 1
                    nc.tensor.matmul(ops[:, 0:gw], act_sh[:, tb2, :],
                                     sw2_sb[:, go:go + gw],
                                     start=False, stop=True)
                    oo = ptmp.tile([128, 512], RS2_DT, name=f"oo_{g}_{tb2}",
                                   tag="dout", bufs=3)
                    if tb2 % 2 == 0:
                        nc.vector.tensor_copy(oo[:, 0:gw], ops[:, 0:gw])
                    else:
                        nc.scalar.activation(oo[:, 0:gw], ops[:, 0:gw],
                                             ACTF.Copy)
                    nc.gpsimd.dma_start(
                        rs2_in[g][tb2 * 128:(tb2 + 1) * 128, :],
                        oo[:, 0:gw])
                nc.gpsimd.collective_compute(
                    "AllToAll", ALU.bypass, replica_groups=rg,
                    ins=[rs2_in[g].opt()], outs=[a2a2_out[g].opt()])

            # local 8-way adds per column group, write f32 output directly
            for g in range(len(W2G)):
                gw, go = W2G[g], W2OFF[g]
                acc = psA.tile([128, 512], F32, name=f"acc_{g}", tag="A")
                for hf in range(2):
                    pg = ptmp.tile([128, 4, 512], RS2_DT,
                                   name=f"opart_{g}_{hf}", tag="pgh",
                                   bufs=2)
                    nc.sync.dma_start(
                        pg[:, :, 0:gw],
                        a2a2_out[g][hf * 4:(hf + 1) * 4].rearrange(
                            "a p b -> p a b"))
                    for i in range(4):
                        nc.tensor.matmul(acc[:, 0:gw], ident_b[:],
                                         pg[:, i, 0:gw],
                                         start=(hf == 0 and i == 0),
                                         stop=(hf == 1 and i == 3))
                og = ptmp.tile([128, 512], F32, name=f"og_{g}", tag="rope1",
                               bufs=2)
                nc.vector.tensor_copy(og[:, 0:gw], acc[:, 0:gw])
                nc.gpsimd.dma_start(out0_d[:, go:go + gw], og[:, 0:gw])

    nc.compile()
    return nc


def prep_in_maps(inputs):
    """Shard/marshal full inputs into 8 per-core input maps."""
    f32 = np.float32
    hid = np.asarray(inputs["hidden_states"], f32)
    res = np.asarray(inputs["residual"], f32)
    rms1 = np.asarray(inputs["rms1_w"], f32)
    rms2 = np.asarray(inputs["rms2_w"], f32)
    w_qkv = np.asarray(inputs["w_qkv"], f32) * rms1[:, None]
    w_dense = np.asarray(inputs["w_dense"], f32)
    gate_w = np.asarray(inputs["gate_w"], f32) * rms2[:, None]
    w13 = np.asarray(inputs["w13"], f32) * rms2[None, :, None]
    w2 = np.asarray(inputs["w2"], f32)
    sw13 = np.asarray(inputs["sw13"], f32) * rms2[:, None]
    sw2 = np.asarray(inputs["sw2"], f32)
    pos = np.asarray(inputs["position_ids"]).astype(f32)

    inv_freq = (1.0 / (ROPE_THETA **
                       (np.arange(0, D, 2, dtype=f32) / D))).astype(f32)
    ang = pos[:, None] * inv_freq[None, :]          # [T, 64]
    cosa = np.cos(ang).astype(f32)
    sina = np.sin(ang).astype(f32)
    s = np.float32(D ** -0.5)

    gate_dev = np.ascontiguousarray(
        gate_w.reshape(HC, 128, E).transpose(1, 0, 2))

    def bf(x):
        return np.ascontiguousarray(x.astype(BF16_NP))

    wkv_dev = bf(w_qkv[:, Q_SIZE:].reshape(HC, 128, 1024))
    wq_dev = bf(w_qkv[:, :Q_SIZE].reshape(HC, 128, Q_SIZE))
    wd_dev = bf(w_dense.reshape(HC, 128, H))

    in_maps = []
    for c in range(N_CORES):
        tok = slice(128 * c, 128 * c + 128)
        co = np.tile(cosa[tok], (1, 8)).reshape(128, 8, 64)
        si = np.tile(sina[tok], (1, 8)).reshape(128, 8, 64)
        causal = (np.arange(T)[None, :]
                  <= (128 * c + np.arange(128))[:, None])
        maskb = np.where(causal, -4.0, -60.0).astype(f32)

        sw13_c = np.concatenate(
            [sw13[:, 128 * c:128 * c + 128],
             sw13[:, SI + 128 * c:SI + 128 * c + 128]], 1)  # [2048, 256]
        sw13_dev = bf(sw13_c.reshape(HC, 128, 256).transpose(1, 0, 2))
        sw2_dev = bf(sw2[128 * c:128 * c + 128, :])       # [128, 2048]

        def w13_dev(e):
            m = w13[e]                                    # [2048, 1024]
            return bf(m.reshape(HC, 128, 8, 128).transpose(2, 1, 0, 3)
                      .reshape(8, 128, H))

        def w2_dev(e):
            m = w2[e]                                     # [512, 2048]
            return bf(m.reshape(4, 128, H).transpose(1, 0, 2))

        sel = np.zeros((2, E, 128), f32)
        sel[0, 2 * c, :] = 1.0
        sel[1, 2 * c + 1, :] = 1.0

        in_maps.append({
            "resid_own": np.ascontiguousarray(hid[tok] + res[tok]),
            "wkv": wkv_dev, "wq": wq_dev, "wdense": wd_dev,
            "sw13": sw13_dev, "sw2": sw2_dev,
            "w13a": w13_dev(2 * c), "w13b": w13_dev(2 * c + 1),
            "w2a": w2_dev(2 * c), "w2b": w2_dev(2 * c + 1),
            "gate": gate_dev,
            "qcos": np.ascontiguousarray(co * s),
            "qsin": np.ascontiguousarray(si * s),
            "kcos": np.ascontiguousarray(co[:, 0:4, :]),
            "ksin": np.ascontiguousarray(si[:, 0:4, :]),
            "maskb": bf(maskb),
            "sela": np.ascontiguousarray(sel[0]),
            "selb": np.ascontiguousarray(sel[1]),
        })
    return in_maps


_NC_CACHE = {}


def _get_nc(debug=False):
    key = debug
    if key not in _NC_CACHE:
        _NC_CACHE[key] = build_nc(debug=debug)
    return _NC_CACHE[key]


def run(inputs, debug=False, trace=False):
    nc = _get_nc(debug=debug)
    in_maps = prep_in_maps(inputs)
    kw = {}
    if trace:
        kw["trace"] = True
    res = run_bass_kernel_spmd(nc, in_maps, core_ids=list(range(N_CORES)),
                               **kw)
    out0 = np.concatenate([res.results[c]["out0"] for c in range(N_CORES)], 0)
    out1 = np.concatenate([res.results[c]["out1"] for c in range(N_CORES)], 0)
    return (out0, out1), res


def kernel(**inputs):
    (out0, out1), _ = run(inputs)
    return out0, out1
